# revision 11
# baseline (speedup 1.0000x reference)
"""Distributed Trainium2 Bass kernel for pre-LN multi-head attention.

Reference computation (per batch b of 2, seq n=2048, dim=1024, 16 heads x 64):
    xn = LayerNorm(x) * gamma + beta
    q, k = split(xn @ W_qk); v = xn @ W_v
    out = softmax(q k^T / 8) v  (per head)
    y = out @ W_out + b_out

Sharding: 8 cores = 2 batch groups x 4 sequence quarters. Core i owns batch
g=i//4, query tokens [qq*512, (qq+1)*512) with qq=i%4. Each core computes
LN + Q/K/V projections for its own 512 tokens, AllGathers K^T and V across
its 4-core group (full 2048-token K/V per batch), runs attention for its 512
queries over all 2048 keys (all 16 heads), and applies the output projection
with the full W_out — so the final output needs no inter-core reduction.
Host assembles the 8 per-core [1024, 512] y^T shards into [2, 2048, 1024].

Optimizations vs the v1 kernel:
- Host packs W_qk into separate K-half / Q-half tensors and gamma/beta/b_out
  into one [128, 24] block, so every weight reaches SBUF in one large
  contiguous-line DMA (no 4-byte gather packets).
- Input DMA order = consumption order: x first (feeds LN), then the K-half
  and W_v (feed the collective payload), Q-half and W_out last. Staging
  writes to the collective input buffers ride the Scalar HWDGE queue so
  they never sit behind weight loads on the Sync queue.
- A dummy 256-byte AllGather is issued at kernel start so the one-time CC
  mesh setup/rendezvous (~37us) overlaps the input DMA + LayerNorm instead
  of delaying the first real gather.
- A short dummy-matmul chain warms the PE clock before real work arrives.
- K/V projections run head-group-major; each group's AllGather triggers as
  soon as its 512KB payload is staged (~30us vs ~67us before).
- Gathered V is stored per key-chunk as [ones(64) | V_h0..h15], so every
  head's AV stationary is a uniform strided AP [ones | V_h] (colsum rows on
  PSUM partitions 0:64, data on 64:128 for all heads) and each (group,
  quarter)'s V lands with one 3-level DMA with 512B lines; the K^T loads are
  merged to two DMAs per (group, quarter). 12 DMAs per group vs 48 before.
"""
import sys
import types

sys.path.insert(0, "/opt/trn_rl_repo")

# Register the NTFF profile hook that trn_boot skips when the image's antenv
# lacks axon_hooks, so run_bass_kernel_spmd(trace=True) can report exec time.
if "antenv.axon_hooks" not in sys.modules:
    try:
        from trn_agent_boot.trn_boot import _ntff_profile_via_ctypes

        _hook = _ntff_profile_via_ctypes("/opt/axon/libaxon_pjrt.so")
    except Exception:
        _hook = None
    _mod = types.ModuleType("antenv.axon_hooks")
    _mod.get_axon_ntff_profile_hook = lambda: _hook
    _mod.set_axon_ntff_profile_hook = lambda h: None
    sys.modules["antenv.axon_hooks"] = _mod

from contextlib import ExitStack

import ml_dtypes
import numpy as np
import concourse.bass as bass
import concourse.tile as tile
from concourse import bacc, mybir
from concourse.bass_utils import run_bass_kernel_spmd
from concourse.masks import make_identity

F32 = mybir.dt.float32
BF16 = mybir.dt.bfloat16
AF = mybir.ActivationFunctionType
ALU = mybir.AluOpType

B, N, DIM = 2, 2048, 1024
HEADS, DH = 16, 64
INNER = HEADS * DH  # 1024
SCALE = DH**-0.5
EPS = 1e-5

NCORES = 8
GROUP = 4          # cores per batch group (sequence quarters)
NQ = N // GROUP    # 512 local query tokens per core
DC = DIM // 128    # 8 dim chunks
KCH = N // 128     # 16 key chunks of 128 tokens
KPAIR = KCH // 2   # exp batches of 2 key chunks

MMDT = BF16        # matmul operand storage dtype

HG_K = 2 * 128 * NQ      # K^T part elements per head-group payload
HG_V = NQ * 256          # V part elements per head-group payload
HG_LEN = HG_K + HG_V
REPLICA_GROUPS = [[0, 1, 2, 3], [4, 5, 6, 7]]

VSTR = 1600              # per-key-chunk vones stride: 8x[ones64|V_2c|V_2c+1] + ones64


def build_nc():
    nc = bacc.Bacc(num_devices=NCORES)

    x = nc.dram_tensor("x", [NQ, DIM], F32, kind="ExternalInput")
    gbb = nc.dram_tensor("gbb", [128, 24], F32, kind="ExternalInput")
    wqk_k = nc.dram_tensor("wqk_k", [DIM, INNER], MMDT, kind="ExternalInput")
    wqk_q = nc.dram_tensor("wqk_q", [DIM, INNER], MMDT, kind="ExternalInput")
    w_v = nc.dram_tensor("W_v", [DIM, INNER], MMDT, kind="ExternalInput")
    w_out = nc.dram_tensor("W_out", [INNER, DIM], MMDT, kind="ExternalInput")
    out = nc.dram_tensor("out", [DIM, NQ], F32, kind="ExternalOutput")

    with tile.TileContext(nc) as tc, ExitStack() as ctx:
        pool = lambda name, bufs, **kw: ctx.enter_context(
            tc.tile_pool(name=name, bufs=bufs, **kw)
        )
        consts = pool("consts", 1)
        dram = pool("dram", 1, space="DRAM")
        qt_pool = pool("qt", 1)
        att_pool = pool("att", 1)
        small = pool("small", 8)
        stage = pool("stage", 3)
        pp = pool("pp", 2, space="PSUM")      # proj / outproj accumulators

        # ---- constants ---------------------------------------------------
        gbb_t = consts.tile([128, 24], F32)   # [gamma | beta | b_out] per c
        nc.sync.dma_start(gbb_t[:], gbb[:, :])
        ident = consts.tile([128, 128], MMDT)
        make_identity(nc, ident[:])
        eps_sb = consts.tile([128, 1], F32)
        nc.vector.memset(eps_sb[:], EPS)
        zsm = consts.tile([128, 1], MMDT)
        nc.vector.memset(zsm[:], 0.0)

        # Dummy 256B AllGather issued first: absorbs the one-time CC mesh
        # rendezvous/setup so the real gathers start transferring on arrival.
        dummy_in = dram.tile([128], MMDT, name="dummy_in")
        dummy_out = dram.tile([GROUP * 128], MMDT, name="dummy_out")
        nc.scalar.dma_start(dummy_in[:].rearrange("(p f) -> p f", f=1), zsm[:])
        nc.gpsimd.collective_compute(
            "AllGather",
            ALU.bypass,
            replica_groups=REPLICA_GROUPS,
            ins=[dummy_in[:].opt()],
            outs=[dummy_out[:].opt()],
        )

        # PE warmup: ramp the clock while input DMAs are in flight.
        wps = pp.tile([128, 512], F32, tag="acc", name="warmup")
        for i in range(24):
            nc.tensor.matmul(
                wps[:, 0:128], ident[:], ident[:], start=(i == 0), stop=(i == 23)
            )

        cc_ins = []
        cc_outs = []
        for hg in range(4):
            cc_i = dram.tile([HG_LEN], MMDT, name=f"cc_in{hg}")
            cc_o = dram.tile([GROUP * HG_LEN], MMDT, name=f"cc_out{hg}")
            cc_ins.append(cc_i)
            cc_outs.append(cc_o)

        # Q^T duplicated per head: head h at cols h*512, rows 0:64 and
        # 64:128 both hold Q_h^T (so S^T matmuls contract over K=128,
        # computing 2*S — folded into the exp scale; K=64 matmuls were
        # observed to hold the HAM clock gate at 1.2 GHz).
        q_t = qt_pool.tile([128, HEADS * NQ], MMDT)
        # attention output^T [1024, 512], chunk c holds heads 2c, 2c+1
        att_t = att_pool.tile([128, DC * NQ], MMDT)

        with ExitStack() as proj_ctx:
            ppool = lambda name, bufs, **kw: proj_ctx.enter_context(
                tc.tile_pool(name=name, bufs=bufs, **kw)
            )
            ptr = ppool("ptr", 2, space="PSUM")  # transpose targets
            xw = ppool("xw", 1)
            x_sb = xw.tile([128, GROUP * DIM], F32)
            xn_nat = xw.tile([128, GROUP * DIM], MMDT)
            xnt = xw.tile([128, DC * NQ], MMDT)
            wk_sb = xw.tile([128, DC * INNER], MMDT)
            wv_sb = xw.tile([128, DC * INNER], MMDT)
            wq_sb = xw.tile([128, DC * INNER], MMDT)

            # Input loads in consumption order, all on the Sync queue.
            for t in range(GROUP):
                nc.sync.dma_start(
                    x_sb[:, t * DIM : (t + 1) * DIM],
                    x[t * 128 : (t + 1) * 128, :],
                )
            # one DMA per weight matrix: [128, c, 1024] with 2KB lines
            nc.sync.dma_start(
                wk_sb[:].rearrange("p (c d) -> p c d", c=DC),
                wqk_k.rearrange("(c p) d -> p c d", p=128),
            )
            nc.sync.dma_start(
                wv_sb[:].rearrange("p (c d) -> p c d", c=DC),
                w_v.rearrange("(c p) d -> p c d", p=128),
            )
            nc.sync.dma_start(
                wq_sb[:].rearrange("p (c d) -> p c d", c=DC),
                wqk_q.rearrange("(c p) d -> p c d", p=128),
            )

            # ---- LayerNorm on the 4 local token chunks ------------------
            for t in range(GROUP):
                xt = x_sb[:, t * DIM : (t + 1) * DIM]
                xg = xt.rearrange("p (n s) -> p n s", s=512)
                stats = small.tile([128, 2, 6], F32)
                for sgi in range(2):
                    nc.vector.bn_stats(stats[:, sgi, :], xg[:, sgi, :])
                mv = small.tile([128, 2], F32)
                nc.vector.bn_aggr(mv[:], stats[:])
                rstd = small.tile([128, 1], F32)
                nc.scalar.activation(rstd[:], mv[:, 1:2], AF.Sqrt, bias=eps_sb[:])
                nc.vector.reciprocal(rstd[:], rstd[:])
                nc.vector.tensor_scalar(
                    out=xn_nat[:, t * DIM : (t + 1) * DIM],
                    in0=xt,
                    scalar1=mv[:, 0:1],
                    scalar2=rstd[:],
                    op0=ALU.subtract,
                    op1=ALU.mult,
                )

            # ---- transpose xn to [dim, tokens], fusing gamma/beta -------
            # split the scale/cast between Vector and Scalar engines
            for c in range(DC):
                for t in range(GROUP):
                    pt = ptr.tile([128, 128], MMDT)
                    nc.tensor.transpose(
                        pt[:],
                        xn_nat[:, t * DIM + c * 128 : t * DIM + (c + 1) * 128],
                        ident[:],
                    )
                    dst = xnt[:, c * NQ + t * 128 : c * NQ + (t + 1) * 128]
                    if t % 2 == 0:
                        nc.vector.tensor_scalar(
                            out=dst,
                            in0=pt[:],
                            scalar1=gbb_t[:, c : c + 1],
                            scalar2=gbb_t[:, 8 + c : 9 + c],
                            op0=ALU.mult,
                            op1=ALU.add,
                        )
                    else:
                        nc.scalar.activation(
                            dst,
                            pt[:],
                            AF.Identity,
                            bias=gbb_t[:, 8 + c : 9 + c],
                            scale=gbb_t[:, c : c + 1],
                        )

            # ---- per head-group: K^T + V projections, then its AllGather -
            for hg in range(4):
                for mg in range(2):
                    m = 2 * hg + mg  # K^T row block (wqk_k col block)
                    pq = pp.tile([128, 512], F32, tag="acc")
                    for c in range(DC):
                        nc.tensor.matmul(
                            pq[:],
                            wk_sb[:, c * INNER + m * 128 : c * INNER + (m + 1) * 128],
                            xnt[:, c * NQ : (c + 1) * NQ],
                            start=(c == 0),
                            stop=(c == DC - 1),
                        )
                    kst = stage.tile([128, 512], MMDT, tag="stg")
                    nc.scalar.copy(kst[:], pq[:])
                    koff = mg * 128 * NQ
                    nc.scalar.dma_start(
                        cc_ins[hg][koff : koff + 128 * NQ].rearrange(
                            "(p f) -> p f", f=NQ
                        ),
                        kst[:],
                    )
                for t in range(GROUP):
                    pv = pp.tile([128, 512], F32, tag="acc")
                    for c in range(DC):
                        nc.tensor.matmul(
                            pv[:, 0:256],
                            xnt[:, c * NQ + t * 128 : c * NQ + (t + 1) * 128],
                            wv_sb[:, c * INNER + hg * 256 : c * INNER + (hg + 1) * 256],
                            start=(c == 0),
                            stop=(c == DC - 1),
                        )
                    vst = stage.tile([128, 512], MMDT, tag="stg")
                    nc.vector.tensor_copy(vst[:, 0:256], pv[:, 0:256])
                    voff = HG_K + t * 128 * 256
                    nc.scalar.dma_start(
                        cc_ins[hg][voff : voff + 128 * 256].rearrange(
                            "(p f) -> p f", f=256
                        ),
                        vst[:, 0:256],
                    )
                nc.gpsimd.collective_compute(
                    "AllGather",
                    ALU.bypass,
                    replica_groups=REPLICA_GROUPS,
                    ins=[cc_ins[hg][:].opt()],
                    outs=[cc_outs[hg][:].opt()],
                )

            # ---- Q^T projection, overlaps the AllGathers ----------------
            for m in range(DC):
                pq = pp.tile([128, 512], F32, tag="acc")
                for c in range(DC):
                    nc.tensor.matmul(
                        pq[:],
                        wq_sb[:, c * INNER + m * 128 : c * INNER + (m + 1) * 128],
                        xnt[:, c * NQ : (c + 1) * NQ],
                        start=(c == 0),
                        stop=(c == DC - 1),
                    )
                for lh in range(2):
                    h_abs = 2 * m + lh
                    for half in range(2):
                        dst = q_t[
                            half * 64 : half * 64 + 64,
                            h_abs * NQ : (h_abs + 1) * NQ,
                        ]
                        src = pq[lh * 64 : lh * 64 + 64, :]
                        if half == 0:
                            nc.vector.tensor_copy(dst, src)
                        else:
                            nc.scalar.copy(dst, src)

        # ---- attention-phase SBUF (proj pools released) ------------------
        with ExitStack() as att_ctx:
            apool = lambda name, bufs, **kw: att_ctx.enter_context(
                tc.tile_pool(name=name, bufs=bufs, **kw)
            )
            kv = apool("kv", 1)
            wo_pool = apool("wo", 1)
            es_pool = apool("es", 18)
            rp_pool = apool("rp", 2)
            y_pool = apool("y", 2)
            ps_s = apool("ps_s", 3, space="PSUM")

            # gathered K^T duplicated per head: quarter qb, head h at cols
            # (qb*16 + h)*512, with K_h^T in both row halves (see q_t note)
            kt_sb = kv.tile([128, GROUP * HEADS * NQ], MMDT)
            # gathered V interleaved with ones blocks: chunk kc spans
            # [kc*1600, +1600): pair c = h//2 at [c*192, +192) as
            # [ones | V_{2c} | V_{2c+1}], plus a trailing ones block.
            # Head h's lhsT = cols kc*1600 + c*192 + (h%2)*128, len 128:
            # even heads [ones | V] (AV rows 0:64 = colsum, 64:128 = data),
            # odd heads [V | ones] (swapped).
            vones = kv.tile([128, KCH * VSTR], MMDT)

            for kc in range(KCH):
                ones_base = vones[:, kc * VSTR : kc * VSTR + 64]
                nc.vector.memset(
                    bass.AP(
                        tensor=ones_base.tensor,
                        offset=ones_base.offset,
                        ap=[ones_base.ap[0], [192, DC + 1], [1, 64]],
                    ),
                    1.0,
                )

            wout_sb = wo_pool.tile([128, DC * DIM], MMDT)
            nc.sync.dma_start(
                wout_sb[:].rearrange("p (c d) -> p c d", c=DC),
                w_out.rearrange("(c p) d -> p c d", p=128),
            )

            # per head-group loads, in attention consumption order;
            # group 0 now, later groups interleaved with the attention loop
            def emit_loads(hg):
                for qb in range(GROUP):
                    # K^T: heads 4hg..4hg+3 merged; same 256-row source into
                    # both destination halves (the K=128 duplication).
                    ksrc = bass.AP(
                        tensor=cc_outs[hg].tensor,
                        offset=cc_outs[hg].offset + qb * HG_LEN,
                        ap=[[NQ, 64], [64 * NQ, 4], [1, NQ]],
                    )
                    for half in range(2):
                        nc.sync.dma_start(
                            kt_sb[
                                half * 64 : half * 64 + 64,
                                (qb * HEADS + 4 * hg) * NQ : (qb * HEADS + 4 * hg + 4)
                                * NQ,
                            ],
                            ksrc,
                        )
                    # V: this quarter's 4 key chunks, one DMA per head pair
                    for pc in range(2):
                        vdst0 = vones[
                            :, qb * 4 * VSTR + (2 * hg + pc) * 192 + 64 :
                        ]
                        nc.sync.dma_start(
                            bass.AP(
                                tensor=vdst0.tensor,
                                offset=vdst0.offset,
                                ap=[vdst0.ap[0], [VSTR, 4], [1, 128]],
                            ),
                            bass.AP(
                                tensor=cc_outs[hg].tensor,
                                offset=cc_outs[hg].offset
                                + qb * HG_LEN
                                + HG_K
                                + pc * 128,
                                ap=[[256, 128], [128 * 256, 4], [1, 128]],
                            ),
                        )
            emit_loads(0)

            def av_lhs(h, kc):
                base = kc * VSTR + (h // 2) * 192 + (h % 2) * 128
                return vones[:, base : base + 128]

            # ---- attention: per head, 16 key chunks in pairs -------------
            # The first two heads of each head-group run "scores-ahead":
            # all S^T/exp pairs are emitted before any AV, so the PE/ACT
            # pipeline advances while the group's V loads drain.
            def head_scores(h):
                ess = []
                for pr in range(KPAIR):
                    pss = ps_s.tile([128, 1024], F32, tag="pss", name="pss")
                    for j in range(2):
                        kc = 2 * pr + j
                        qb, t4 = kc // 4, kc % 4
                        lhs_k = kt_sb[
                            :,
                            (qb * HEADS + h) * NQ + t4 * 128 : (qb * HEADS + h) * NQ
                            + (t4 + 1) * 128,
                        ]
                        nc.tensor.matmul(
                            pss[:, j * 512 : (j + 1) * 512],
                            lhs_k,
                            q_t[:, h * NQ : (h + 1) * NQ],
                            start=True,
                            stop=True,
                        )
                    es = es_pool.tile([128, 1024], MMDT, tag="es", name="es")
                    # psum holds 2*S (duplicated operands) -> halve the scale
                    nc.scalar.activation(es[:], pss[:], AF.Exp, scale=SCALE / 2)
                    ess.append(es)
                return ess

            def head_avs(h, ess):
                po = pp.tile([128, 512], F32, tag="acc", name="po")
                for pr in range(KPAIR):
                    for j in range(2):
                        kc = 2 * pr + j
                        nc.tensor.matmul(
                            po[:],
                            av_lhs(h, kc),
                            ess[pr][:, j * 512 : (j + 1) * 512],
                            start=(pr == 0 and j == 0),
                            stop=(pr == KPAIR - 1 and j == 1),
                        )
                return po

            def head_divide(h, po):
                hp = (h % 2) * 64
                hc = h // 2
                cb, dp = hp, 64 - hp
                recip = rp_pool.tile([128, 1024], F32, tag="recip", name="recip")
                nc.vector.tensor_copy(recip[0:64, 512:1024], po[cb : cb + 64, :])
                nc.vector.reciprocal_approx_fast(
                    recip[0:64, 0:512], recip[0:64, 512:1024]
                )
                nc.vector.tensor_mul(
                    att_t[hp : hp + 64, hc * NQ : (hc + 1) * NQ],
                    po[dp : dp + 64, :],
                    recip[0:64, 0:512],
                )

            for g in range(4):
                h0, h1 = 4 * g, 4 * g + 1
                ess0 = head_scores(h0)
                ess1 = head_scores(h1)
                po0 = head_avs(h0, ess0)
                head_divide(h0, po0)
                po1 = head_avs(h1, ess1)
                head_divide(h1, po1)
                if g < 3:
                    emit_loads(g + 1)
                for h in (4 * g + 2, 4 * g + 3):
                    # normal interleaved pipeline for the rest of the group
                    po = pp.tile([128, 512], F32, tag="acc", name="po")
                    for pr in range(KPAIR):
                        pss = ps_s.tile([128, 1024], F32, tag="pss", name="pss")
                        for j in range(2):
                            kc = 2 * pr + j
                            qb, t4 = kc // 4, kc % 4
                            lhs_k = kt_sb[
                                :,
                                (qb * HEADS + h) * NQ + t4 * 128 : (qb * HEADS + h)
                                * NQ
                                + (t4 + 1) * 128,
                            ]
                            nc.tensor.matmul(
                                pss[:, j * 512 : (j + 1) * 512],
                                lhs_k,
                                q_t[:, h * NQ : (h + 1) * NQ],
                                start=True,
                                stop=True,
                            )
                        es = es_pool.tile([128, 1024], MMDT, tag="es", name="es")
                        nc.scalar.activation(es[:], pss[:], AF.Exp, scale=SCALE / 2)
                        for j in range(2):
                            kc = 2 * pr + j
                            nc.tensor.matmul(
                                po[:],
                                av_lhs(h, kc),
                                es[:, j * 512 : (j + 1) * 512],
                                start=(pr == 0 and j == 0),
                                stop=(pr == KPAIR - 1 and j == 1),
                            )
                    head_divide(h, po)

            # ---- output projection y^T = W_out^T @ att^T + b_out ---------
            for m in range(DC):
                py = pp.tile([128, 512], F32, tag="acc")
                for c in range(DC):
                    nc.tensor.matmul(
                        py[:],
                        wout_sb[:, c * DIM + m * 128 : c * DIM + (m + 1) * 128],
                        att_t[:, c * NQ : (c + 1) * NQ],
                        start=(c == 0),
                        stop=(c == DC - 1),
                    )
                y_sb = y_pool.tile([128, 512], F32, tag="y")
                nc.vector.tensor_scalar(
                    out=y_sb[:],
                    in0=py[:],
                    scalar1=gbb_t[:, 16 + m : 17 + m],
                    scalar2=None,
                    op0=ALU.add,
                )
                nc.sync.dma_start(out[m * 128 : (m + 1) * 128, :], y_sb[:])

    nc.compile()
    return nc


_NC_CACHE = None


def _get_nc():
    global _NC_CACHE
    if _NC_CACHE is None:
        _NC_CACHE = build_nc()
    return _NC_CACHE


def _make_in_maps(x, ln_gamma, ln_beta, W_qk, W_v, W_out, b_out):
    mmnp = mybir.dt.np(MMDT)
    wqk = np.asarray(W_qk, dtype=np.float32)
    wqk_q = np.ascontiguousarray(wqk[:, :INNER]).astype(mmnp)
    wqk_k = np.ascontiguousarray(wqk[:, INNER:]).astype(mmnp)
    wv = np.ascontiguousarray(np.asarray(W_v, dtype=np.float32)).astype(mmnp)
    wo = np.ascontiguousarray(np.asarray(W_out, dtype=np.float32)).astype(mmnp)
    gamma = np.asarray(ln_gamma, dtype=np.float32).reshape(DC, 128).T
    beta = np.asarray(ln_beta, dtype=np.float32).reshape(DC, 128).T
    bout = np.asarray(b_out, dtype=np.float32).reshape(DC, 128).T
    gbb = np.ascontiguousarray(np.concatenate([gamma, beta, bout], axis=1))
    xf = np.asarray(x, dtype=np.float32)
    in_maps = []
    for i in range(NCORES):
        g, qq = i // GROUP, i % GROUP
        in_maps.append(
            {
                "x": np.ascontiguousarray(xf[g, qq * NQ : (qq + 1) * NQ, :]),
                "gbb": gbb,
                "wqk_k": wqk_k,
                "wqk_q": wqk_q,
                "W_v": wv,
                "W_out": wo,
            }
        )
    return in_maps


def run(inputs: dict, trace: bool = False):
    """Run the distributed kernel; returns (full_output, BassKernelResults)."""
    nc = _get_nc()
    in_maps = _make_in_maps(**inputs)
    res = run_bass_kernel_spmd(
        nc, in_maps, core_ids=list(range(NCORES)), trace=trace
    )
    out_full = np.empty((B, N, DIM), dtype=np.float32)
    for i in range(NCORES):
        g, qq = i // GROUP, i % GROUP
        out_full[g, qq * NQ : (qq + 1) * NQ, :] = res.results[i]["out"].T
    return out_full, res


def kernel(**inputs) -> np.ndarray:
    out, _ = run(inputs, trace=False)
    return out


# revision 14
# speedup vs baseline: 1.0334x; 1.0334x over previous
"""Distributed Trainium2 Bass kernel for pre-LN multi-head attention.

Reference computation (per batch b of 2, seq n=2048, dim=1024, 16 heads x 64):
    xn = LayerNorm(x) * gamma + beta
    q, k = split(xn @ W_qk); v = xn @ W_v
    out = softmax(q k^T / 8) v  (per head)
    y = out @ W_out + b_out

Sharding: 8 cores = 2 batch groups x 4 sequence quarters. Core i owns batch
g=i//4, query tokens [qq*512, (qq+1)*512) with qq=i%4. Each core computes
LN + Q/K/V projections for its own 512 tokens, AllGathers K^T and V across
its 4-core group (full 2048-token K/V per batch), runs attention for its 512
queries over all 2048 keys (all 16 heads), and applies the output projection
with the full W_out — so the final output needs no inter-core reduction.
Host assembles the 8 per-core [1024, 512] y^T shards into [2, 2048, 1024].

Optimizations vs the v1 kernel:
- Host packs W_qk into separate K-half / Q-half tensors and gamma/beta/b_out
  into one [128, 24] block, so every weight reaches SBUF in one large
  contiguous-line DMA (no 4-byte gather packets).
- Input DMA order = consumption order: x first (feeds LN), then the K-half
  and W_v (feed the collective payload), Q-half and W_out last. Staging
  writes to the collective input buffers ride the Scalar HWDGE queue so
  they never sit behind weight loads on the Sync queue.
- A dummy 256-byte AllGather is issued at kernel start so the one-time CC
  mesh setup/rendezvous (~37us) overlaps the input DMA + LayerNorm instead
  of delaying the first real gather.
- A short dummy-matmul chain warms the PE clock before real work arrives.
- K/V projections run head-group-major; each group's AllGather triggers as
  soon as its 512KB payload is staged (~30us vs ~67us before).
- Gathered V is stored per key-chunk as [ones(64) | V_h0..h15], so every
  head's AV stationary is a uniform strided AP [ones | V_h] (colsum rows on
  PSUM partitions 0:64, data on 64:128 for all heads) and each (group,
  quarter)'s V lands with one 3-level DMA with 512B lines; the K^T loads are
  merged to two DMAs per (group, quarter). 12 DMAs per group vs 48 before.
"""
import sys
import types

sys.path.insert(0, "/opt/trn_rl_repo")

# Register the NTFF profile hook that trn_boot skips when the image's antenv
# lacks axon_hooks, so run_bass_kernel_spmd(trace=True) can report exec time.
if "antenv.axon_hooks" not in sys.modules:
    try:
        from trn_agent_boot.trn_boot import _ntff_profile_via_ctypes

        _hook = _ntff_profile_via_ctypes("/opt/axon/libaxon_pjrt.so")
    except Exception:
        _hook = None
    _mod = types.ModuleType("antenv.axon_hooks")
    _mod.get_axon_ntff_profile_hook = lambda: _hook
    _mod.set_axon_ntff_profile_hook = lambda h: None
    sys.modules["antenv.axon_hooks"] = _mod

from contextlib import ExitStack

import ml_dtypes
import numpy as np
import concourse.bass as bass
import concourse.tile as tile
from concourse import bacc, mybir
from concourse.bass_utils import run_bass_kernel_spmd
from concourse.masks import make_identity

F32 = mybir.dt.float32
BF16 = mybir.dt.bfloat16
AF = mybir.ActivationFunctionType
ALU = mybir.AluOpType

B, N, DIM = 2, 2048, 1024
HEADS, DH = 16, 64
INNER = HEADS * DH  # 1024
SCALE = DH**-0.5
EPS = 1e-5

NCORES = 8
GROUP = 4          # cores per batch group (sequence quarters)
NQ = N // GROUP    # 512 local query tokens per core
DC = DIM // 128    # 8 dim chunks
KCH = N // 128     # 16 key chunks of 128 tokens
KPAIR = KCH // 2   # exp batches of 2 key chunks

MMDT = BF16        # matmul operand storage dtype

HG_K = 2 * 128 * NQ      # K^T part elements per head-group payload
HG_V = NQ * 256          # V part elements per head-group payload
HG_LEN = HG_K + HG_V
REPLICA_GROUPS = [[0, 1, 2, 3], [4, 5, 6, 7]]

VSTR = 1600              # per-key-chunk vones stride: 8x[ones64|V_2c|V_2c+1] + ones64


def build_nc():
    nc = bacc.Bacc(num_devices=NCORES)

    x = nc.dram_tensor("x", [NQ, DIM], F32, kind="ExternalInput")
    gbb = nc.dram_tensor("gbb", [128, 24], F32, kind="ExternalInput")
    wqk_k = nc.dram_tensor("wqk_k", [DIM, INNER], MMDT, kind="ExternalInput")
    wqk_q = nc.dram_tensor("wqk_q", [DIM, INNER], MMDT, kind="ExternalInput")
    w_v = nc.dram_tensor("W_v", [DIM, INNER], MMDT, kind="ExternalInput")
    w_out = nc.dram_tensor("W_out", [INNER, DIM], MMDT, kind="ExternalInput")
    out = nc.dram_tensor("out", [DIM, NQ], F32, kind="ExternalOutput")

    with tile.TileContext(nc) as tc, ExitStack() as ctx:
        pool = lambda name, bufs, **kw: ctx.enter_context(
            tc.tile_pool(name=name, bufs=bufs, **kw)
        )
        consts = pool("consts", 1)
        dram = pool("dram", 1, space="DRAM")
        qt_pool = pool("qt", 1)
        att_pool = pool("att", 1)
        small = pool("small", 8)
        stage = pool("stage", 3)
        pp = pool("pp", 2, space="PSUM")      # proj / outproj accumulators

        # ---- constants ---------------------------------------------------
        gbb_t = consts.tile([128, 24], F32)   # [gamma | beta | b_out] per c
        nc.sync.dma_start(gbb_t[:], gbb[:, :])
        ident = consts.tile([128, 128], MMDT)
        make_identity(nc, ident[:])
        eps_sb = consts.tile([128, 1], F32)
        nc.vector.memset(eps_sb[:], EPS)
        # PE warmup: ramp the clock while input DMAs are in flight.
        wps = pp.tile([128, 512], F32, tag="acc", name="warmup")
        for i in range(24):
            nc.tensor.matmul(
                wps[:, 0:128], ident[:], ident[:], start=(i == 0), stop=(i == 23)
            )

        cc_ins = []
        cc_outs = []
        for hg in range(4):
            cc_i = dram.tile([HG_LEN], MMDT, name=f"cc_in{hg}")
            cc_o = dram.tile([GROUP * HG_LEN], MMDT, name=f"cc_out{hg}")
            cc_ins.append(cc_i)
            cc_outs.append(cc_o)

        # Q^T duplicated per head: head h at cols h*512, rows 0:64 and
        # 64:128 both hold Q_h^T (so S^T matmuls contract over K=128,
        # computing 2*S — folded into the exp scale; K=64 matmuls were
        # observed to hold the HAM clock gate at 1.2 GHz).
        q_t = qt_pool.tile([128, HEADS * NQ], MMDT)
        # attention output^T [1024, 512], chunk c holds heads 2c, 2c+1
        att_t = att_pool.tile([128, DC * NQ], MMDT)

        with ExitStack() as proj_ctx:
            ppool = lambda name, bufs, **kw: proj_ctx.enter_context(
                tc.tile_pool(name=name, bufs=bufs, **kw)
            )
            ptr = ppool("ptr", 2, space="PSUM")  # transpose targets
            xw = ppool("xw", 1)
            x_sb = xw.tile([128, GROUP * DIM], F32)
            xn_nat = xw.tile([128, GROUP * DIM], MMDT)
            xnt = xw.tile([128, DC * NQ], MMDT)
            wk_sb = xw.tile([128, DC * INNER], MMDT)
            wv_sb = xw.tile([128, DC * INNER], MMDT)
            wq_sb = xw.tile([128, DC * INNER], MMDT)

            # Input loads in consumption order, all on the Sync queue.
            # 256KB per DMA: bigger single DMAs stall the HWDGE ring and
            # starve packet dispatch for everything queued behind them.
            for t in range(GROUP):
                nc.sync.dma_start(
                    x_sb[:, t * DIM : (t + 1) * DIM],
                    x[t * 128 : (t + 1) * 128, :],
                )
            for c in range(DC):
                nc.sync.dma_start(
                    wk_sb[:, c * INNER : (c + 1) * INNER],
                    wqk_k[c * 128 : (c + 1) * 128, :],
                )
            for c in range(DC):
                nc.sync.dma_start(
                    wv_sb[:, c * INNER : (c + 1) * INNER],
                    w_v[c * 128 : (c + 1) * 128, :],
                )
            for c in range(DC):
                nc.sync.dma_start(
                    wq_sb[:, c * INNER : (c + 1) * INNER],
                    wqk_q[c * 128 : (c + 1) * 128, :],
                )

            # ---- LayerNorm on the 4 local token chunks ------------------
            for t in range(GROUP):
                xt = x_sb[:, t * DIM : (t + 1) * DIM]
                xg = xt.rearrange("p (n s) -> p n s", s=512)
                stats = small.tile([128, 2, 6], F32)
                for sgi in range(2):
                    nc.vector.bn_stats(stats[:, sgi, :], xg[:, sgi, :])
                mv = small.tile([128, 2], F32)
                nc.vector.bn_aggr(mv[:], stats[:])
                rstd = small.tile([128, 1], F32)
                nc.scalar.activation(rstd[:], mv[:, 1:2], AF.Sqrt, bias=eps_sb[:])
                nc.vector.reciprocal(rstd[:], rstd[:])
                nc.vector.tensor_scalar(
                    out=xn_nat[:, t * DIM : (t + 1) * DIM],
                    in0=xt,
                    scalar1=mv[:, 0:1],
                    scalar2=rstd[:],
                    op0=ALU.subtract,
                    op1=ALU.mult,
                )

            # ---- transpose xn to [dim, tokens], fusing gamma/beta -------
            # split the scale/cast between Vector and Scalar engines
            for c in range(DC):
                for t in range(GROUP):
                    pt = ptr.tile([128, 128], MMDT)
                    nc.tensor.transpose(
                        pt[:],
                        xn_nat[:, t * DIM + c * 128 : t * DIM + (c + 1) * 128],
                        ident[:],
                    )
                    dst = xnt[:, c * NQ + t * 128 : c * NQ + (t + 1) * 128]
                    if t % 2 == 0:
                        nc.vector.tensor_scalar(
                            out=dst,
                            in0=pt[:],
                            scalar1=gbb_t[:, c : c + 1],
                            scalar2=gbb_t[:, 8 + c : 9 + c],
                            op0=ALU.mult,
                            op1=ALU.add,
                        )
                    else:
                        nc.scalar.activation(
                            dst,
                            pt[:],
                            AF.Identity,
                            bias=gbb_t[:, 8 + c : 9 + c],
                            scale=gbb_t[:, c : c + 1],
                        )

            # ---- per head-group: K^T + V projections, then its AllGather -
            for hg in range(4):
                for mg in range(2):
                    m = 2 * hg + mg  # K^T row block (wqk_k col block)
                    pq = pp.tile([128, 512], F32, tag="acc")
                    for c in range(DC):
                        nc.tensor.matmul(
                            pq[:],
                            wk_sb[:, c * INNER + m * 128 : c * INNER + (m + 1) * 128],
                            xnt[:, c * NQ : (c + 1) * NQ],
                            start=(c == 0),
                            stop=(c == DC - 1),
                        )
                    kst = stage.tile([128, 512], MMDT, tag="stg")
                    nc.scalar.copy(kst[:], pq[:])
                    koff = mg * 128 * NQ
                    nc.scalar.dma_start(
                        cc_ins[hg][koff : koff + 128 * NQ].rearrange(
                            "(p f) -> p f", f=NQ
                        ),
                        kst[:],
                    )
                for t in range(GROUP):
                    pv = pp.tile([128, 512], F32, tag="acc")
                    for c in range(DC):
                        nc.tensor.matmul(
                            pv[:, 0:256],
                            xnt[:, c * NQ + t * 128 : c * NQ + (t + 1) * 128],
                            wv_sb[:, c * INNER + hg * 256 : c * INNER + (hg + 1) * 256],
                            start=(c == 0),
                            stop=(c == DC - 1),
                        )
                    vst = stage.tile([128, 512], MMDT, tag="stg")
                    nc.vector.tensor_copy(vst[:, 0:256], pv[:, 0:256])
                    voff = HG_K + t * 128 * 256
                    nc.scalar.dma_start(
                        cc_ins[hg][voff : voff + 128 * 256].rearrange(
                            "(p f) -> p f", f=256
                        ),
                        vst[:, 0:256],
                    )
                nc.gpsimd.collective_compute(
                    "AllGather",
                    ALU.bypass,
                    replica_groups=REPLICA_GROUPS,
                    ins=[cc_ins[hg][:].opt()],
                    outs=[cc_outs[hg][:].opt()],
                )

            # ---- Q^T projection, overlaps the AllGathers ----------------
            for m in range(DC):
                pq = pp.tile([128, 512], F32, tag="acc")
                for c in range(DC):
                    nc.tensor.matmul(
                        pq[:],
                        wq_sb[:, c * INNER + m * 128 : c * INNER + (m + 1) * 128],
                        xnt[:, c * NQ : (c + 1) * NQ],
                        start=(c == 0),
                        stop=(c == DC - 1),
                    )
                for lh in range(2):
                    h_abs = 2 * m + lh
                    for half in range(2):
                        dst = q_t[
                            half * 64 : half * 64 + 64,
                            h_abs * NQ : (h_abs + 1) * NQ,
                        ]
                        src = pq[lh * 64 : lh * 64 + 64, :]
                        if half == 0:
                            nc.vector.tensor_copy(dst, src)
                        else:
                            nc.scalar.copy(dst, src)

        # ---- attention-phase SBUF (proj pools released) ------------------
        with ExitStack() as att_ctx:
            apool = lambda name, bufs, **kw: att_ctx.enter_context(
                tc.tile_pool(name=name, bufs=bufs, **kw)
            )
            kv = apool("kv", 1)
            wo_pool = apool("wo", 1)
            es_pool = apool("es", 18)
            rp_pool = apool("rp", 2)
            y_pool = apool("y", 2)
            ps_s = apool("ps_s", 3, space="PSUM")

            # gathered K^T duplicated per head: quarter qb, head h at cols
            # (qb*16 + h)*512, with K_h^T in both row halves (see q_t note)
            kt_sb = kv.tile([128, GROUP * HEADS * NQ], MMDT)
            # gathered V interleaved with ones blocks: chunk kc spans
            # [kc*1600, +1600): pair c = h//2 at [c*192, +192) as
            # [ones | V_{2c} | V_{2c+1}], plus a trailing ones block.
            # Head h's lhsT = cols kc*1600 + c*192 + (h%2)*128, len 128:
            # even heads [ones | V] (AV rows 0:64 = colsum, 64:128 = data),
            # odd heads [V | ones] (swapped).
            vones = kv.tile([128, KCH * VSTR], MMDT)

            for kc in range(KCH):
                ones_base = vones[:, kc * VSTR : kc * VSTR + 64]
                nc.vector.memset(
                    bass.AP(
                        tensor=ones_base.tensor,
                        offset=ones_base.offset,
                        ap=[ones_base.ap[0], [192, DC + 1], [1, 64]],
                    ),
                    1.0,
                )

            # W_out rides the Scalar queue: the Sync queue must stay free
            # for the latency-critical kt/vones loads after each gather.
            wout_sb = wo_pool.tile([128, DC * DIM], MMDT)
            for c in range(DC):
                nc.scalar.dma_start(
                    wout_sb[:, c * DIM : (c + 1) * DIM],
                    w_out[c * 128 : (c + 1) * 128, :],
                )

            # per head-group loads, in attention consumption order;
            # group 0 now, later groups interleaved with the attention loop
            def emit_loads(hg):
                for qb in range(GROUP):
                    # K^T: heads 4hg..4hg+3 merged; same 256-row source into
                    # both destination halves (the K=128 duplication).
                    ksrc = bass.AP(
                        tensor=cc_outs[hg].tensor,
                        offset=cc_outs[hg].offset + qb * HG_LEN,
                        ap=[[NQ, 64], [64 * NQ, 4], [1, NQ]],
                    )
                    for half in range(2):
                        nc.sync.dma_start(
                            kt_sb[
                                half * 64 : half * 64 + 64,
                                (qb * HEADS + 4 * hg) * NQ : (qb * HEADS + 4 * hg + 4)
                                * NQ,
                            ],
                            ksrc,
                        )
                    # V: this quarter's 4 key chunks, one DMA per head pair
                    for pc in range(2):
                        vdst0 = vones[
                            :, qb * 4 * VSTR + (2 * hg + pc) * 192 + 64 :
                        ]
                        nc.sync.dma_start(
                            bass.AP(
                                tensor=vdst0.tensor,
                                offset=vdst0.offset,
                                ap=[vdst0.ap[0], [VSTR, 4], [1, 128]],
                            ),
                            bass.AP(
                                tensor=cc_outs[hg].tensor,
                                offset=cc_outs[hg].offset
                                + qb * HG_LEN
                                + HG_K
                                + pc * 128,
                                ap=[[256, 128], [128 * 256, 4], [1, 128]],
                            ),
                        )
            emit_loads(0)

            def av_lhs(h, kc):
                base = kc * VSTR + (h // 2) * 192 + (h % 2) * 128
                return vones[:, base : base + 128]

            # ---- attention: per head, 16 key chunks in pairs -------------
            # The first two heads of each head-group run "scores-ahead":
            # all S^T/exp pairs are emitted before any AV, so the PE/ACT
            # pipeline advances while the group's V loads drain.
            def head_scores(h):
                ess = []
                for pr in range(KPAIR):
                    pss = ps_s.tile([128, 1024], F32, tag="pss", name="pss")
                    for j in range(2):
                        kc = 2 * pr + j
                        qb, t4 = kc // 4, kc % 4
                        lhs_k = kt_sb[
                            :,
                            (qb * HEADS + h) * NQ + t4 * 128 : (qb * HEADS + h) * NQ
                            + (t4 + 1) * 128,
                        ]
                        nc.tensor.matmul(
                            pss[:, j * 512 : (j + 1) * 512],
                            lhs_k,
                            q_t[:, h * NQ : (h + 1) * NQ],
                            start=True,
                            stop=True,
                        )
                    es = es_pool.tile([128, 1024], MMDT, tag="es", name="es")
                    # psum holds 2*S (duplicated operands) -> halve the scale
                    nc.scalar.activation(es[:], pss[:], AF.Exp, scale=SCALE / 2)
                    ess.append(es)
                return ess

            def head_avs(h, ess):
                po = pp.tile([128, 512], F32, tag="acc", name="po")
                for pr in range(KPAIR):
                    for j in range(2):
                        kc = 2 * pr + j
                        nc.tensor.matmul(
                            po[:],
                            av_lhs(h, kc),
                            ess[pr][:, j * 512 : (j + 1) * 512],
                            start=(pr == 0 and j == 0),
                            stop=(pr == KPAIR - 1 and j == 1),
                        )
                return po

            def head_divide(h, po):
                hp = (h % 2) * 64
                hc = h // 2
                cb, dp = hp, 64 - hp
                recip = rp_pool.tile([128, 1024], F32, tag="recip", name="recip")
                nc.vector.tensor_copy(recip[0:64, 512:1024], po[cb : cb + 64, :])
                nc.vector.reciprocal_approx_fast(
                    recip[0:64, 0:512], recip[0:64, 512:1024]
                )
                nc.vector.tensor_mul(
                    att_t[hp : hp + 64, hc * NQ : (hc + 1) * NQ],
                    po[dp : dp + 64, :],
                    recip[0:64, 0:512],
                )

            for g in range(4):
                h0, h1 = 4 * g, 4 * g + 1
                ess0 = head_scores(h0)
                ess1 = head_scores(h1)
                po0 = head_avs(h0, ess0)
                head_divide(h0, po0)
                po1 = head_avs(h1, ess1)
                head_divide(h1, po1)
                if g < 3:
                    emit_loads(g + 1)
                for h in (4 * g + 2, 4 * g + 3):
                    # normal interleaved pipeline for the rest of the group
                    po = pp.tile([128, 512], F32, tag="acc", name="po")
                    for pr in range(KPAIR):
                        pss = ps_s.tile([128, 1024], F32, tag="pss", name="pss")
                        for j in range(2):
                            kc = 2 * pr + j
                            qb, t4 = kc // 4, kc % 4
                            lhs_k = kt_sb[
                                :,
                                (qb * HEADS + h) * NQ + t4 * 128 : (qb * HEADS + h)
                                * NQ
                                + (t4 + 1) * 128,
                            ]
                            nc.tensor.matmul(
                                pss[:, j * 512 : (j + 1) * 512],
                                lhs_k,
                                q_t[:, h * NQ : (h + 1) * NQ],
                                start=True,
                                stop=True,
                            )
                        es = es_pool.tile([128, 1024], MMDT, tag="es", name="es")
                        nc.scalar.activation(es[:], pss[:], AF.Exp, scale=SCALE / 2)
                        for j in range(2):
                            kc = 2 * pr + j
                            nc.tensor.matmul(
                                po[:],
                                av_lhs(h, kc),
                                es[:, j * 512 : (j + 1) * 512],
                                start=(pr == 0 and j == 0),
                                stop=(pr == KPAIR - 1 and j == 1),
                            )
                    head_divide(h, po)

            # ---- output projection y^T = W_out^T @ att^T + b_out ---------
            for m in range(DC):
                py = pp.tile([128, 512], F32, tag="acc")
                for c in range(DC):
                    nc.tensor.matmul(
                        py[:],
                        wout_sb[:, c * DIM + m * 128 : c * DIM + (m + 1) * 128],
                        att_t[:, c * NQ : (c + 1) * NQ],
                        start=(c == 0),
                        stop=(c == DC - 1),
                    )
                y_sb = y_pool.tile([128, 512], F32, tag="y")
                nc.vector.tensor_scalar(
                    out=y_sb[:],
                    in0=py[:],
                    scalar1=gbb_t[:, 16 + m : 17 + m],
                    scalar2=None,
                    op0=ALU.add,
                )
                nc.sync.dma_start(out[m * 128 : (m + 1) * 128, :], y_sb[:])

    nc.compile()
    return nc


_NC_CACHE = None


def _get_nc():
    global _NC_CACHE
    if _NC_CACHE is None:
        _NC_CACHE = build_nc()
    return _NC_CACHE


def _make_in_maps(x, ln_gamma, ln_beta, W_qk, W_v, W_out, b_out):
    mmnp = mybir.dt.np(MMDT)
    wqk = np.asarray(W_qk, dtype=np.float32)
    wqk_q = np.ascontiguousarray(wqk[:, :INNER]).astype(mmnp)
    wqk_k = np.ascontiguousarray(wqk[:, INNER:]).astype(mmnp)
    wv = np.ascontiguousarray(np.asarray(W_v, dtype=np.float32)).astype(mmnp)
    wo = np.ascontiguousarray(np.asarray(W_out, dtype=np.float32)).astype(mmnp)
    gamma = np.asarray(ln_gamma, dtype=np.float32).reshape(DC, 128).T
    beta = np.asarray(ln_beta, dtype=np.float32).reshape(DC, 128).T
    bout = np.asarray(b_out, dtype=np.float32).reshape(DC, 128).T
    gbb = np.ascontiguousarray(np.concatenate([gamma, beta, bout], axis=1))
    xf = np.asarray(x, dtype=np.float32)
    in_maps = []
    for i in range(NCORES):
        g, qq = i // GROUP, i % GROUP
        in_maps.append(
            {
                "x": np.ascontiguousarray(xf[g, qq * NQ : (qq + 1) * NQ, :]),
                "gbb": gbb,
                "wqk_k": wqk_k,
                "wqk_q": wqk_q,
                "W_v": wv,
                "W_out": wo,
            }
        )
    return in_maps


def run(inputs: dict, trace: bool = False):
    """Run the distributed kernel; returns (full_output, BassKernelResults)."""
    nc = _get_nc()
    in_maps = _make_in_maps(**inputs)
    res = run_bass_kernel_spmd(
        nc, in_maps, core_ids=list(range(NCORES)), trace=trace
    )
    out_full = np.empty((B, N, DIM), dtype=np.float32)
    for i in range(NCORES):
        g, qq = i // GROUP, i % GROUP
        out_full[g, qq * NQ : (qq + 1) * NQ, :] = res.results[i]["out"].T
    return out_full, res


def kernel(**inputs) -> np.ndarray:
    out, _ = run(inputs, trace=False)
    return out


# revision 19
# speedup vs baseline: 1.0976x; 1.0621x over previous
"""Distributed Trainium2 Bass kernel for pre-LN multi-head attention.

Reference computation (per batch b of 2, seq n=2048, dim=1024, 16 heads x 64):
    xn = LayerNorm(x) * gamma + beta
    q, k = split(xn @ W_qk); v = xn @ W_v
    out = softmax(q k^T / 8) v  (per head)
    y = out @ W_out + b_out

Sharding: 8 cores = 2 batch groups x 4 sequence quarters. Core i owns batch
g=i//4, query tokens [qq*512, (qq+1)*512) with qq=i%4. Each core computes
LN + Q/K/V projections for its own 512 tokens, AllGathers K^T and V across
its 4-core group (full 2048-token K/V per batch), runs attention for its 512
queries over all 2048 keys (all 16 heads), and applies the output projection
with the full W_out — so the final output needs no inter-core reduction.
Host assembles the 8 per-core [1024, 512] y^T shards into [2, 2048, 1024].

Optimizations vs the v1 kernel:
- Host packs W_qk into separate K-half / Q-half tensors and gamma/beta/b_out
  into one [128, 24] block, so every weight reaches SBUF in one large
  contiguous-line DMA (no 4-byte gather packets).
- Input DMA order = consumption order: x first (feeds LN), then the K-half
  and W_v (feed the collective payload), Q-half and W_out last. Staging
  writes to the collective input buffers ride the Scalar HWDGE queue so
  they never sit behind weight loads on the Sync queue.
- A dummy 256-byte AllGather is issued at kernel start so the one-time CC
  mesh setup/rendezvous (~37us) overlaps the input DMA + LayerNorm instead
  of delaying the first real gather.
- A short dummy-matmul chain warms the PE clock before real work arrives.
- K/V projections run head-group-major; each group's AllGather triggers as
  soon as its 512KB payload is staged (~30us vs ~67us before).
- Gathered V is stored per key-chunk as [ones(64) | V_h0..h15], so every
  head's AV stationary is a uniform strided AP [ones | V_h] (colsum rows on
  PSUM partitions 0:64, data on 64:128 for all heads) and each (group,
  quarter)'s V lands with one 3-level DMA with 512B lines; the K^T loads are
  merged to two DMAs per (group, quarter). 12 DMAs per group vs 48 before.
"""
import sys
import types

sys.path.insert(0, "/opt/trn_rl_repo")

# Register the NTFF profile hook that trn_boot skips when the image's antenv
# lacks axon_hooks, so run_bass_kernel_spmd(trace=True) can report exec time.
if "antenv.axon_hooks" not in sys.modules:
    try:
        from trn_agent_boot.trn_boot import _ntff_profile_via_ctypes

        _hook = _ntff_profile_via_ctypes("/opt/axon/libaxon_pjrt.so")
    except Exception:
        _hook = None
    _mod = types.ModuleType("antenv.axon_hooks")
    _mod.get_axon_ntff_profile_hook = lambda: _hook
    _mod.set_axon_ntff_profile_hook = lambda h: None
    sys.modules["antenv.axon_hooks"] = _mod

from contextlib import ExitStack

import ml_dtypes
import numpy as np
import concourse.bass as bass
import concourse.tile as tile
from concourse import bacc, mybir
from concourse.bass_utils import run_bass_kernel_spmd
from concourse.masks import make_identity

F32 = mybir.dt.float32
BF16 = mybir.dt.bfloat16
AF = mybir.ActivationFunctionType
ALU = mybir.AluOpType

B, N, DIM = 2, 2048, 1024
HEADS, DH = 16, 64
INNER = HEADS * DH  # 1024
SCALE = DH**-0.5
EPS = 1e-5

NCORES = 8
GROUP = 4          # cores per batch group (sequence quarters)
NQ = N // GROUP    # 512 local query tokens per core
DC = DIM // 128    # 8 dim chunks
KCH = N // 128     # 16 key chunks of 128 tokens
KPAIR = KCH // 2   # exp batches of 2 key chunks

MMDT = BF16        # matmul operand storage dtype

HG_K = 2 * 128 * NQ      # K^T part elements per head-group payload
HG_V = NQ * 256          # V part elements per head-group payload
HG_LEN = HG_K + HG_V
REPLICA_GROUPS = [[0, 1, 2, 3], [4, 5, 6, 7]]

VSTR = 1600              # per-key-chunk vones stride: 8x[ones64|V_2c|V_2c+1] + ones64


def build_nc():
    nc = bacc.Bacc(num_devices=NCORES)

    x = nc.dram_tensor("x", [NQ, DIM], F32, kind="ExternalInput")
    gbb = nc.dram_tensor("gbb", [128, 24], F32, kind="ExternalInput")
    wqk_k = nc.dram_tensor("wqk_k", [DIM, INNER], MMDT, kind="ExternalInput")
    wqk_q = nc.dram_tensor("wqk_q", [DIM, INNER], MMDT, kind="ExternalInput")
    w_v = nc.dram_tensor("W_v", [DIM, INNER], MMDT, kind="ExternalInput")
    w_out = nc.dram_tensor("W_out", [INNER, DIM], MMDT, kind="ExternalInput")
    out = nc.dram_tensor("out", [DIM, NQ], F32, kind="ExternalOutput")

    with tile.TileContext(nc) as tc, ExitStack() as ctx:
        pool = lambda name, bufs, **kw: ctx.enter_context(
            tc.tile_pool(name=name, bufs=bufs, **kw)
        )
        consts = pool("consts", 1)
        dram = pool("dram", 1, space="DRAM")
        qt_pool = pool("qt", 1)
        att_pool = pool("att", 1)
        small = pool("small", 8)
        stage = pool("stage", 3)
        pp = pool("pp", 2, space="PSUM")      # proj / outproj accumulators

        # ---- constants ---------------------------------------------------
        gbb_t = consts.tile([128, 24], F32)   # [gamma | beta | b_out] per c
        nc.sync.dma_start(gbb_t[:], gbb[:, :])
        ident = consts.tile([128, 128], MMDT)
        make_identity(nc, ident[:])
        eps_sb = consts.tile([128, 1], F32)
        nc.vector.memset(eps_sb[:], EPS)
        # PE warmup: ramp the clock while input DMAs are in flight.
        wps = pp.tile([128, 512], F32, tag="acc", name="warmup")
        for i in range(24):
            nc.tensor.matmul(
                wps[:, 0:128], ident[:], ident[:], start=(i == 0), stop=(i == 23)
            )

        cc_ins = []
        cc_outs = []
        for hg in range(4):
            cc_i = dram.tile([HG_LEN], MMDT, name=f"cc_in{hg}")
            cc_o = dram.tile([GROUP * HG_LEN], MMDT, name=f"cc_out{hg}")
            cc_ins.append(cc_i)
            cc_outs.append(cc_o)

        # Q^T duplicated per head: head h at cols h*512, rows 0:64 and
        # 64:128 both hold Q_h^T (so S^T matmuls contract over K=128,
        # computing 2*S — folded into the exp scale; K=64 matmuls were
        # observed to hold the HAM clock gate at 1.2 GHz).
        q_t = qt_pool.tile([128, HEADS * NQ], MMDT)
        # attention output^T [1024, 512], chunk c holds heads 2c, 2c+1
        att_t = att_pool.tile([128, DC * NQ], MMDT)

        with ExitStack() as proj_ctx:
            ppool = lambda name, bufs, **kw: proj_ctx.enter_context(
                tc.tile_pool(name=name, bufs=bufs, **kw)
            )
            ptr = ppool("ptr", 2, space="PSUM")  # transpose targets
            pkv = ppool("pkv", 2, space="PSUM")  # second accumulation chain
            xw = ppool("xw", 1)
            x_sb = xw.tile([128, GROUP * DIM], F32)
            xn_nat = xw.tile([128, GROUP * DIM], MMDT)
            xnt = xw.tile([128, DC * NQ], MMDT)
            wk_sb = xw.tile([128, DC * INNER], MMDT)
            wv_sb = xw.tile([128, DC * INNER], MMDT)
            wq_sb = xw.tile([128, DC * INNER], MMDT)

            # Input loads in consumption order, all on the Sync queue.
            # 256KB per DMA: bigger single DMAs stall the HWDGE ring and
            # starve packet dispatch for everything queued behind them.
            for t in range(GROUP):
                nc.sync.dma_start(
                    x_sb[:, t * DIM : (t + 1) * DIM],
                    x[t * 128 : (t + 1) * 128, :],
                )
            for c in range(DC):
                nc.sync.dma_start(
                    wk_sb[:, c * INNER : (c + 1) * INNER],
                    wqk_k[c * 128 : (c + 1) * 128, :],
                )
            for c in range(DC):
                nc.sync.dma_start(
                    wv_sb[:, c * INNER : (c + 1) * INNER],
                    w_v[c * 128 : (c + 1) * 128, :],
                )
            for c in range(DC):
                nc.sync.dma_start(
                    wq_sb[:, c * INNER : (c + 1) * INNER],
                    wqk_q[c * 128 : (c + 1) * 128, :],
                )

            # ---- LayerNorm on the 4 local token chunks ------------------
            for t in range(GROUP):
                xt = x_sb[:, t * DIM : (t + 1) * DIM]
                xg = xt.rearrange("p (n s) -> p n s", s=512)
                stats = small.tile([128, 2, 6], F32)
                for sgi in range(2):
                    nc.vector.bn_stats(stats[:, sgi, :], xg[:, sgi, :])
                mv = small.tile([128, 2], F32)
                nc.vector.bn_aggr(mv[:], stats[:])
                rstd = small.tile([128, 1], F32)
                nc.scalar.activation(rstd[:], mv[:, 1:2], AF.Sqrt, bias=eps_sb[:])
                nc.vector.reciprocal(rstd[:], rstd[:])
                nc.vector.tensor_scalar(
                    out=xn_nat[:, t * DIM : (t + 1) * DIM],
                    in0=xt,
                    scalar1=mv[:, 0:1],
                    scalar2=rstd[:],
                    op0=ALU.subtract,
                    op1=ALU.mult,
                )

            # ---- transpose xn to [dim, tokens], fusing gamma/beta -------
            # split the scale/cast between Vector and Scalar engines
            for c in range(DC):
                for t in range(GROUP):
                    pt = ptr.tile([128, 128], MMDT)
                    nc.tensor.transpose(
                        pt[:],
                        xn_nat[:, t * DIM + c * 128 : t * DIM + (c + 1) * 128],
                        ident[:],
                    )
                    dst = xnt[:, c * NQ + t * 128 : c * NQ + (t + 1) * 128]
                    if t % 2 == 0:
                        nc.vector.tensor_scalar(
                            out=dst,
                            in0=pt[:],
                            scalar1=gbb_t[:, c : c + 1],
                            scalar2=gbb_t[:, 8 + c : 9 + c],
                            op0=ALU.mult,
                            op1=ALU.add,
                        )
                    else:
                        nc.scalar.activation(
                            dst,
                            pt[:],
                            AF.Identity,
                            bias=gbb_t[:, 8 + c : 9 + c],
                            scale=gbb_t[:, c : c + 1],
                        )

            # ---- per head-group: K^T + V projections, then its AllGather -
            # Two accumulation chains run interleaved (separate PSUM banks):
            # back-to-back matmuls into one accumulator serialize (~720ns
            # each), interleaved independent chains pipeline (~2x faster).
            for hg in range(4):
                pq0 = pp.tile([128, 512], F32, tag="acc")
                pq1 = pkv.tile([128, 512], F32, tag="kvacc")
                pqs = [pq0, pq1]
                for c in range(DC):
                    for mg in range(2):
                        m = 2 * hg + mg  # K^T row block (wqk_k col block)
                        nc.tensor.matmul(
                            pqs[mg][:],
                            wk_sb[:, c * INNER + m * 128 : c * INNER + (m + 1) * 128],
                            xnt[:, c * NQ : (c + 1) * NQ],
                            start=(c == 0),
                            stop=(c == DC - 1),
                        )
                for mg in range(2):
                    kst = stage.tile([128, 512], MMDT, tag="stg")
                    nc.scalar.copy(kst[:], pqs[mg][:])
                    koff = mg * 128 * NQ
                    nc.scalar.dma_start(
                        cc_ins[hg][koff : koff + 128 * NQ].rearrange(
                            "(p f) -> p f", f=NQ
                        ),
                        kst[:],
                    )
                for tp in range(2):
                    pv0 = pp.tile([128, 512], F32, tag="acc")
                    pv1 = pkv.tile([128, 512], F32, tag="kvacc")
                    pvs = [pv0, pv1]
                    for c in range(DC):
                        for ti in range(2):
                            t = 2 * tp + ti
                            nc.tensor.matmul(
                                pvs[ti][:, 0:256],
                                xnt[:, c * NQ + t * 128 : c * NQ + (t + 1) * 128],
                                wv_sb[
                                    :, c * INNER + hg * 256 : c * INNER + (hg + 1) * 256
                                ],
                                start=(c == 0),
                                stop=(c == DC - 1),
                            )
                    for ti in range(2):
                        t = 2 * tp + ti
                        vst = stage.tile([128, 512], MMDT, tag="stg")
                        nc.vector.tensor_copy(vst[:, 0:256], pvs[ti][:, 0:256])
                        voff = HG_K + t * 128 * 256
                        nc.scalar.dma_start(
                            cc_ins[hg][voff : voff + 128 * 256].rearrange(
                                "(p f) -> p f", f=256
                            ),
                            vst[:, 0:256],
                        )
                nc.gpsimd.collective_compute(
                    "AllGather",
                    ALU.bypass,
                    replica_groups=REPLICA_GROUPS,
                    ins=[cc_ins[hg][:].opt()],
                    outs=[cc_outs[hg][:].opt()],
                )

            # ---- Q^T projection, overlaps the AllGathers ----------------
            for mp in range(DC // 2):
                pq0 = pp.tile([128, 512], F32, tag="acc")
                pq1 = pkv.tile([128, 512], F32, tag="kvacc")
                pqs = [pq0, pq1]
                for c in range(DC):
                    for mi in range(2):
                        m = 2 * mp + mi
                        nc.tensor.matmul(
                            pqs[mi][:],
                            wq_sb[:, c * INNER + m * 128 : c * INNER + (m + 1) * 128],
                            xnt[:, c * NQ : (c + 1) * NQ],
                            start=(c == 0),
                            stop=(c == DC - 1),
                        )
                for mi in range(2):
                    m = 2 * mp + mi
                    for lh in range(2):
                        h_abs = 2 * m + lh
                        for half in range(2):
                            dst = q_t[
                                half * 64 : half * 64 + 64,
                                h_abs * NQ : (h_abs + 1) * NQ,
                            ]
                            src = pqs[mi][lh * 64 : lh * 64 + 64, :]
                            if half == 0:
                                nc.vector.tensor_copy(dst, src)
                            else:
                                nc.scalar.copy(dst, src)

        # ---- attention-phase SBUF (proj pools released) ------------------
        with ExitStack() as att_ctx:
            apool = lambda name, bufs, **kw: att_ctx.enter_context(
                tc.tile_pool(name=name, bufs=bufs, **kw)
            )
            kv = apool("kv", 1)
            wo_pool = apool("wo", 1)
            es_pool = apool("es", 18)
            rp_pool = apool("rp", 2)
            y_pool = apool("y", 2)
            ps_s = apool("ps_s", 3, space="PSUM")

            # gathered K^T duplicated per head: quarter qb, head h at cols
            # (qb*16 + h)*512, with K_h^T in both row halves (see q_t note)
            kt_sb = kv.tile([128, GROUP * HEADS * NQ], MMDT)
            # gathered V interleaved with ones blocks: chunk kc spans
            # [kc*1600, +1600): pair c = h//2 at [c*192, +192) as
            # [ones | V_{2c} | V_{2c+1}], plus a trailing ones block.
            # Head h's lhsT = cols kc*1600 + c*192 + (h%2)*128, len 128:
            # even heads [ones | V] (AV rows 0:64 = colsum, 64:128 = data),
            # odd heads [V | ones] (swapped).
            vones = kv.tile([128, KCH * VSTR], MMDT)

            for kc in range(KCH):
                ones_base = vones[:, kc * VSTR : kc * VSTR + 64]
                nc.vector.memset(
                    bass.AP(
                        tensor=ones_base.tensor,
                        offset=ones_base.offset,
                        ap=[ones_base.ap[0], [192, DC + 1], [1, 64]],
                    ),
                    1.0,
                )

            # W_out chunks are DMA'd from inside the attention loop (on the
            # Sync queue, behind the later emit_loads) so they never compete
            # with the AllGathers or the latency-critical kt/vones loads.
            wout_sb = wo_pool.tile([128, DC * DIM], MMDT)

            def emit_wout(cs):
                for c in cs:
                    nc.sync.dma_start(
                        wout_sb[:, c * DIM : (c + 1) * DIM],
                        w_out[c * 128 : (c + 1) * 128, :],
                    )

            # per head-group loads, in attention consumption order;
            # group 0 now, later groups interleaved with the attention loop
            def emit_loads(hg):
                for qb in range(GROUP):
                    # K^T: heads 4hg..4hg+3 merged. Half 0 comes from HBM;
                    # half 1 (the K=128 duplication) is an SBUF->SBUF copy
                    # so it doesn't compete with the AllGathers for HBM.
                    ksrc = bass.AP(
                        tensor=cc_outs[hg].tensor,
                        offset=cc_outs[hg].offset + qb * HG_LEN,
                        ap=[[NQ, 64], [64 * NQ, 4], [1, NQ]],
                    )
                    span = slice(
                        (qb * HEADS + 4 * hg) * NQ, (qb * HEADS + 4 * hg + 4) * NQ
                    )
                    nc.sync.dma_start(kt_sb[0:64, span], ksrc)
                    nc.sync.dma_start(kt_sb[64:128, span], kt_sb[0:64, span])
                    # V: this quarter's 4 key chunks, one DMA per head pair
                    for pc in range(2):
                        vdst0 = vones[
                            :, qb * 4 * VSTR + (2 * hg + pc) * 192 + 64 :
                        ]
                        nc.sync.dma_start(
                            bass.AP(
                                tensor=vdst0.tensor,
                                offset=vdst0.offset,
                                ap=[vdst0.ap[0], [VSTR, 4], [1, 128]],
                            ),
                            bass.AP(
                                tensor=cc_outs[hg].tensor,
                                offset=cc_outs[hg].offset
                                + qb * HG_LEN
                                + HG_K
                                + pc * 128,
                                ap=[[256, 128], [128 * 256, 4], [1, 128]],
                            ),
                        )
            emit_loads(0)

            def av_lhs(h, kc):
                base = kc * VSTR + (h // 2) * 192 + (h % 2) * 128
                return vones[:, base : base + 128]

            # ---- attention: per head, 16 key chunks in pairs -------------
            # The first two heads of each head-group run "scores-ahead":
            # all S^T/exp pairs are emitted before any AV, so the PE/ACT
            # pipeline advances while the group's V loads drain.
            def head_scores(h):
                ess = []
                for pr in range(KPAIR):
                    pss = ps_s.tile([128, 1024], F32, tag="pss", name="pss")
                    for j in range(2):
                        kc = 2 * pr + j
                        qb, t4 = kc // 4, kc % 4
                        lhs_k = kt_sb[
                            :,
                            (qb * HEADS + h) * NQ + t4 * 128 : (qb * HEADS + h) * NQ
                            + (t4 + 1) * 128,
                        ]
                        nc.tensor.matmul(
                            pss[:, j * 512 : (j + 1) * 512],
                            lhs_k,
                            q_t[:, h * NQ : (h + 1) * NQ],
                            start=True,
                            stop=True,
                        )
                    es = es_pool.tile([128, 1024], MMDT, tag="es", name="es")
                    # psum holds 2*S (duplicated operands) -> halve the scale
                    nc.scalar.activation(es[:], pss[:], AF.Exp, scale=SCALE / 2)
                    ess.append(es)
                return ess

            def head_avs(h, ess):
                po = pp.tile([128, 512], F32, tag="acc", name="po")
                for pr in range(KPAIR):
                    for j in range(2):
                        kc = 2 * pr + j
                        nc.tensor.matmul(
                            po[:],
                            av_lhs(h, kc),
                            ess[pr][:, j * 512 : (j + 1) * 512],
                            start=(pr == 0 and j == 0),
                            stop=(pr == KPAIR - 1 and j == 1),
                        )
                return po

            def head_divide(h, po):
                hp = (h % 2) * 64
                hc = h // 2
                cb, dp = hp, 64 - hp
                recip = rp_pool.tile([128, 1024], F32, tag="recip", name="recip")
                nc.vector.tensor_copy(recip[0:64, 512:1024], po[cb : cb + 64, :])
                nc.vector.reciprocal_approx_fast(
                    recip[0:64, 0:512], recip[0:64, 512:1024]
                )
                nc.vector.tensor_mul(
                    att_t[hp : hp + 64, hc * NQ : (hc + 1) * NQ],
                    po[dp : dp + 64, :],
                    recip[0:64, 0:512],
                )

            for g in range(4):
                h0, h1 = 4 * g, 4 * g + 1
                ess0 = head_scores(h0)
                ess1 = head_scores(h1)
                po0 = head_avs(h0, ess0)
                head_divide(h0, po0)
                po1 = head_avs(h1, ess1)
                head_divide(h1, po1)
                if g < 3:
                    emit_loads(g + 1)
                emit_wout([2 * g, 2 * g + 1])
                for h in (4 * g + 2, 4 * g + 3):
                    # normal interleaved pipeline for the rest of the group
                    po = pp.tile([128, 512], F32, tag="acc", name="po")
                    for pr in range(KPAIR):
                        pss = ps_s.tile([128, 1024], F32, tag="pss", name="pss")
                        for j in range(2):
                            kc = 2 * pr + j
                            qb, t4 = kc // 4, kc % 4
                            lhs_k = kt_sb[
                                :,
                                (qb * HEADS + h) * NQ + t4 * 128 : (qb * HEADS + h)
                                * NQ
                                + (t4 + 1) * 128,
                            ]
                            nc.tensor.matmul(
                                pss[:, j * 512 : (j + 1) * 512],
                                lhs_k,
                                q_t[:, h * NQ : (h + 1) * NQ],
                                start=True,
                                stop=True,
                            )
                        es = es_pool.tile([128, 1024], MMDT, tag="es", name="es")
                        nc.scalar.activation(es[:], pss[:], AF.Exp, scale=SCALE / 2)
                        for j in range(2):
                            kc = 2 * pr + j
                            nc.tensor.matmul(
                                po[:],
                                av_lhs(h, kc),
                                es[:, j * 512 : (j + 1) * 512],
                                start=(pr == 0 and j == 0),
                                stop=(pr == KPAIR - 1 and j == 1),
                            )
                    head_divide(h, po)

            # ---- output projection y^T = W_out^T @ att^T + b_out ---------
            for m in range(DC):
                py = pp.tile([128, 512], F32, tag="acc")
                for c in range(DC):
                    nc.tensor.matmul(
                        py[:],
                        wout_sb[:, c * DIM + m * 128 : c * DIM + (m + 1) * 128],
                        att_t[:, c * NQ : (c + 1) * NQ],
                        start=(c == 0),
                        stop=(c == DC - 1),
                    )
                y_sb = y_pool.tile([128, 512], F32, tag="y")
                nc.vector.tensor_scalar(
                    out=y_sb[:],
                    in0=py[:],
                    scalar1=gbb_t[:, 16 + m : 17 + m],
                    scalar2=None,
                    op0=ALU.add,
                )
                nc.sync.dma_start(out[m * 128 : (m + 1) * 128, :], y_sb[:])

    nc.compile()
    return nc


_NC_CACHE = None


def _get_nc():
    global _NC_CACHE
    if _NC_CACHE is None:
        _NC_CACHE = build_nc()
    return _NC_CACHE


def _make_in_maps(x, ln_gamma, ln_beta, W_qk, W_v, W_out, b_out):
    mmnp = mybir.dt.np(MMDT)
    wqk = np.asarray(W_qk, dtype=np.float32)
    wqk_q = np.ascontiguousarray(wqk[:, :INNER]).astype(mmnp)
    wqk_k = np.ascontiguousarray(wqk[:, INNER:]).astype(mmnp)
    wv = np.ascontiguousarray(np.asarray(W_v, dtype=np.float32)).astype(mmnp)
    wo = np.ascontiguousarray(np.asarray(W_out, dtype=np.float32)).astype(mmnp)
    gamma = np.asarray(ln_gamma, dtype=np.float32).reshape(DC, 128).T
    beta = np.asarray(ln_beta, dtype=np.float32).reshape(DC, 128).T
    bout = np.asarray(b_out, dtype=np.float32).reshape(DC, 128).T
    gbb = np.ascontiguousarray(np.concatenate([gamma, beta, bout], axis=1))
    xf = np.asarray(x, dtype=np.float32)
    in_maps = []
    for i in range(NCORES):
        g, qq = i // GROUP, i % GROUP
        in_maps.append(
            {
                "x": np.ascontiguousarray(xf[g, qq * NQ : (qq + 1) * NQ, :]),
                "gbb": gbb,
                "wqk_k": wqk_k,
                "wqk_q": wqk_q,
                "W_v": wv,
                "W_out": wo,
            }
        )
    return in_maps


def run(inputs: dict, trace: bool = False):
    """Run the distributed kernel; returns (full_output, BassKernelResults)."""
    nc = _get_nc()
    in_maps = _make_in_maps(**inputs)
    res = run_bass_kernel_spmd(
        nc, in_maps, core_ids=list(range(NCORES)), trace=trace
    )
    out_full = np.empty((B, N, DIM), dtype=np.float32)
    for i in range(NCORES):
        g, qq = i // GROUP, i % GROUP
        out_full[g, qq * NQ : (qq + 1) * NQ, :] = res.results[i]["out"].T
    return out_full, res


def kernel(**inputs) -> np.ndarray:
    out, _ = run(inputs, trace=False)
    return out


# revision 26
# speedup vs baseline: 1.1169x; 1.0176x over previous
"""Distributed Trainium2 Bass kernel for pre-LN multi-head attention.

Reference computation (per batch b of 2, seq n=2048, dim=1024, 16 heads x 64):
    xn = LayerNorm(x) * gamma + beta
    q, k = split(xn @ W_qk); v = xn @ W_v
    out = softmax(q k^T / 8) v  (per head)
    y = out @ W_out + b_out

Sharding: 8 cores = 2 batch groups x 4 sequence quarters. Core i owns batch
g=i//4, query tokens [qq*512, (qq+1)*512) with qq=i%4. Each core computes
LN + Q/K/V projections for its own 512 tokens, AllGathers K^T and V across
its 4-core group (full 2048-token K/V per batch), runs attention for its 512
queries over all 2048 keys (all 16 heads), and applies the output projection
with the full W_out — so the final output needs no inter-core reduction.
Host assembles the 8 per-core [1024, 512] y^T shards into [2, 2048, 1024].

Optimizations vs the v1 kernel:
- Host packs W_qk into separate K-half / Q-half tensors and gamma/beta/b_out
  into one [128, 24] block, so every weight reaches SBUF in one large
  contiguous-line DMA (no 4-byte gather packets).
- Input DMA order = consumption order: x first (feeds LN), then the K-half
  and W_v (feed the collective payload), Q-half and W_out last. Staging
  writes to the collective input buffers ride the Scalar HWDGE queue so
  they never sit behind weight loads on the Sync queue.
- A dummy 256-byte AllGather is issued at kernel start so the one-time CC
  mesh setup/rendezvous (~37us) overlaps the input DMA + LayerNorm instead
  of delaying the first real gather.
- A short dummy-matmul chain warms the PE clock before real work arrives.
- K/V projections run head-group-major; each group's AllGather triggers as
  soon as its 512KB payload is staged (~30us vs ~67us before).
- Gathered V is stored per key-chunk as [ones(64) | V_h0..h15], so every
  head's AV stationary is a uniform strided AP [ones | V_h] (colsum rows on
  PSUM partitions 0:64, data on 64:128 for all heads) and each (group,
  quarter)'s V lands with one 3-level DMA with 512B lines; the K^T loads are
  merged to two DMAs per (group, quarter). 12 DMAs per group vs 48 before.
"""
import sys
import types

sys.path.insert(0, "/opt/trn_rl_repo")

# Register the NTFF profile hook that trn_boot skips when the image's antenv
# lacks axon_hooks, so run_bass_kernel_spmd(trace=True) can report exec time.
if "antenv.axon_hooks" not in sys.modules:
    try:
        from trn_agent_boot.trn_boot import _ntff_profile_via_ctypes

        _hook = _ntff_profile_via_ctypes("/opt/axon/libaxon_pjrt.so")
    except Exception:
        _hook = None
    _mod = types.ModuleType("antenv.axon_hooks")
    _mod.get_axon_ntff_profile_hook = lambda: _hook
    _mod.set_axon_ntff_profile_hook = lambda h: None
    sys.modules["antenv.axon_hooks"] = _mod

from contextlib import ExitStack

import ml_dtypes
import numpy as np
import concourse.bass as bass
import concourse.tile as tile
from concourse import bacc, mybir
from concourse.bass_utils import run_bass_kernel_spmd
from concourse.masks import make_identity

F32 = mybir.dt.float32
BF16 = mybir.dt.bfloat16
AF = mybir.ActivationFunctionType
ALU = mybir.AluOpType

B, N, DIM = 2, 2048, 1024
HEADS, DH = 16, 64
INNER = HEADS * DH  # 1024
SCALE = DH**-0.5
EPS = 1e-5

NCORES = 8
GROUP = 4          # cores per batch group (sequence quarters)
NQ = N // GROUP    # 512 local query tokens per core
DC = DIM // 128    # 8 dim chunks
KCH = N // 128     # 16 key chunks of 128 tokens
KPAIR = KCH // 2   # exp batches of 2 key chunks

MMDT = BF16        # matmul operand storage dtype

REPLICA_GROUPS = [[0, 1, 2, 3], [4, 5, 6, 7]]

# Gather subgroups: first head-group split 2+2 so the first payload lands
# ~16us earlier and the attention/exp stream starts sooner; the rest in 4s.
SG = [[0, 1], [2, 3], [4, 5, 6, 7], [8, 9, 10, 11], [12, 13, 14, 15]]


def sg_klen(hh):
    return len(hh) * 64 * NQ


def sg_len(hh):
    return 2 * len(hh) * 64 * NQ

VSTR = 1600              # per-key-chunk vones stride: 8x[ones64|V_2c|V_2c+1] + ones64


def build_nc():
    nc = bacc.Bacc(num_devices=NCORES)

    x = nc.dram_tensor("x", [NQ, DIM], F32, kind="ExternalInput")
    gbb = nc.dram_tensor("gbb", [128, 24], F32, kind="ExternalInput")
    wqk_k = nc.dram_tensor("wqk_k", [DIM, INNER], MMDT, kind="ExternalInput")
    wqk_q = nc.dram_tensor("wqk_q", [DIM, INNER], MMDT, kind="ExternalInput")
    w_v = nc.dram_tensor("W_v", [DIM, INNER], MMDT, kind="ExternalInput")
    w_out = nc.dram_tensor("W_out", [INNER, DIM], MMDT, kind="ExternalInput")
    out = nc.dram_tensor("out", [DIM, NQ], F32, kind="ExternalOutput")

    with tile.TileContext(nc) as tc, ExitStack() as ctx:
        pool = lambda name, bufs, **kw: ctx.enter_context(
            tc.tile_pool(name=name, bufs=bufs, **kw)
        )
        consts = pool("consts", 1)
        dram = pool("dram", 1, space="DRAM")
        qt_pool = pool("qt", 1)
        att_pool = pool("att", 1)
        small = pool("small", 8)
        stage = pool("stage", 3)
        pp = pool("pp", 2, space="PSUM")      # proj / outproj accumulators

        # ---- constants ---------------------------------------------------
        gbb_t = consts.tile([128, 24], F32)   # [gamma | beta | b_out] per c
        nc.sync.dma_start(gbb_t[:], gbb[:, :])
        ident = consts.tile([128, 128], MMDT)
        make_identity(nc, ident[:])
        eps_sb = consts.tile([128, 1], F32)
        nc.vector.memset(eps_sb[:], EPS)
        # PE warmup: ramp the clock while input DMAs are in flight.
        wps = pp.tile([128, 512], F32, tag="acc", name="warmup")
        for i in range(24):
            nc.tensor.matmul(
                wps[:, 0:128], ident[:], ident[:], start=(i == 0), stop=(i == 23)
            )

        cc_ins = []
        cc_outs = []
        for s, hh in enumerate(SG):
            cc_i = dram.tile([sg_len(hh)], MMDT, name=f"cc_in{s}")
            cc_o = dram.tile([GROUP * sg_len(hh)], MMDT, name=f"cc_out{s}")
            cc_ins.append(cc_i)
            cc_outs.append(cc_o)

        # Q^T duplicated per head: head h at cols h*512, rows 0:64 and
        # 64:128 both hold Q_h^T (so S^T matmuls contract over K=128,
        # computing 2*S — folded into the exp scale; K=64 matmuls were
        # observed to hold the HAM clock gate at 1.2 GHz).
        q_t = qt_pool.tile([128, HEADS * NQ], MMDT)
        # attention output^T [1024, 512], chunk c holds heads 2c, 2c+1
        att_t = att_pool.tile([128, DC * NQ], MMDT)

        with ExitStack() as proj_ctx:
            ppool = lambda name, bufs, **kw: proj_ctx.enter_context(
                tc.tile_pool(name=name, bufs=bufs, **kw)
            )
            ptr = ppool("ptr", 2, space="PSUM")  # transpose targets
            pkv = ppool("pkv", 2, space="PSUM")  # second accumulation chain
            xw = ppool("xw", 1)
            x_sb = xw.tile([128, GROUP * DIM], F32)
            xn_nat = xw.tile([128, GROUP * DIM], MMDT)
            xnt = xw.tile([128, DC * NQ], MMDT)
            wk_sb = xw.tile([128, DC * INNER], MMDT)
            wv_sb = xw.tile([128, DC * INNER], MMDT)
            wq_sb = xw.tile([128, DC * INNER], MMDT)

            # Input loads in consumption order, all on the Sync queue.
            # 256KB per DMA: bigger single DMAs stall the HWDGE ring and
            # starve packet dispatch for everything queued behind them.
            for t in range(GROUP):
                nc.sync.dma_start(
                    x_sb[:, t * DIM : (t + 1) * DIM],
                    x[t * 128 : (t + 1) * 128, :],
                )
            for c in range(DC):
                nc.sync.dma_start(
                    wk_sb[:, c * INNER : (c + 1) * INNER],
                    wqk_k[c * 128 : (c + 1) * 128, :],
                )
            for c in range(DC):
                nc.sync.dma_start(
                    wv_sb[:, c * INNER : (c + 1) * INNER],
                    w_v[c * 128 : (c + 1) * 128, :],
                )
            for c in range(DC):
                nc.sync.dma_start(
                    wq_sb[:, c * INNER : (c + 1) * INNER],
                    wqk_q[c * 128 : (c + 1) * 128, :],
                )

            # ---- LayerNorm on the 4 local token chunks ------------------
            for t in range(GROUP):
                xt = x_sb[:, t * DIM : (t + 1) * DIM]
                xg = xt.rearrange("p (n s) -> p n s", s=512)
                stats = small.tile([128, 2, 6], F32)
                for sgi in range(2):
                    nc.vector.bn_stats(stats[:, sgi, :], xg[:, sgi, :])
                mv = small.tile([128, 2], F32)
                nc.vector.bn_aggr(mv[:], stats[:])
                rstd = small.tile([128, 1], F32)
                nc.scalar.activation(rstd[:], mv[:, 1:2], AF.Sqrt, bias=eps_sb[:])
                nc.vector.reciprocal(rstd[:], rstd[:])
                nc.vector.tensor_scalar(
                    out=xn_nat[:, t * DIM : (t + 1) * DIM],
                    in0=xt,
                    scalar1=mv[:, 0:1],
                    scalar2=rstd[:],
                    op0=ALU.subtract,
                    op1=ALU.mult,
                )

            # ---- transpose xn to [dim, tokens], fusing gamma/beta -------
            # split the scale/cast between Vector and Scalar engines
            for c in range(DC):
                for t in range(GROUP):
                    pt = ptr.tile([128, 128], MMDT)
                    nc.tensor.transpose(
                        pt[:],
                        xn_nat[:, t * DIM + c * 128 : t * DIM + (c + 1) * 128],
                        ident[:],
                    )
                    dst = xnt[:, c * NQ + t * 128 : c * NQ + (t + 1) * 128]
                    if t % 2 == 0:
                        nc.vector.tensor_scalar(
                            out=dst,
                            in0=pt[:],
                            scalar1=gbb_t[:, c : c + 1],
                            scalar2=gbb_t[:, 8 + c : 9 + c],
                            op0=ALU.mult,
                            op1=ALU.add,
                        )
                    else:
                        nc.scalar.activation(
                            dst,
                            pt[:],
                            AF.Identity,
                            bias=gbb_t[:, 8 + c : 9 + c],
                            scale=gbb_t[:, c : c + 1],
                        )

            # ---- per subgroup: K^T + V projections, then its AllGather ---
            # Two accumulation chains run interleaved (separate PSUM banks):
            # back-to-back matmuls into one accumulator serialize (~720ns
            # each), interleaved independent chains pipeline (~2x faster).
            def k_chains(ms):
                pqs = [
                    (pp if i == 0 else pkv).tile(
                        [128, 512],
                        F32,
                        tag="acc" if i == 0 else "kvacc",
                        name=f"pk{i}",
                    )
                    for i in range(len(ms))
                ]
                for c in range(DC):
                    for i, m in enumerate(ms):
                        nc.tensor.matmul(
                            pqs[i][:],
                            wk_sb[:, c * INNER + m * 128 : c * INNER + (m + 1) * 128],
                            xnt[:, c * NQ : (c + 1) * NQ],
                            start=(c == 0),
                            stop=(c == DC - 1),
                        )
                return pqs

            def stage_k(s, hh, mi, pq):
                kst = stage.tile([128, 512], MMDT, tag="stg")
                nc.scalar.copy(kst[:], pq[:])
                koff = mi * 128 * NQ
                nc.scalar.dma_start(
                    cc_ins[s][koff : koff + 128 * NQ].rearrange("(p f) -> p f", f=NQ),
                    kst[:],
                )

            def stage_v(s, hh):
                vw = len(hh) * 64  # V columns for this subgroup
                for tp in range(2):
                    pvs = [
                        (pp if ti == 0 else pkv).tile(
                            [128, 512],
                            F32,
                            tag="acc" if ti == 0 else "kvacc",
                            name=f"pv{ti}",
                        )
                        for ti in range(2)
                    ]
                    for c in range(DC):
                        for ti in range(2):
                            t = 2 * tp + ti
                            nc.tensor.matmul(
                                pvs[ti][:, 0:vw],
                                xnt[:, c * NQ + t * 128 : c * NQ + (t + 1) * 128],
                                wv_sb[
                                    :,
                                    c * INNER + hh[0] * 64 : c * INNER
                                    + (hh[0] + len(hh)) * 64,
                                ],
                                start=(c == 0),
                                stop=(c == DC - 1),
                            )
                    for ti in range(2):
                        t = 2 * tp + ti
                        vst = stage.tile([128, 512], MMDT, tag="stg")
                        nc.vector.tensor_copy(vst[:, 0:vw], pvs[ti][:, 0:vw])
                        voff = sg_klen(hh) + t * 128 * vw
                        nc.scalar.dma_start(
                            cc_ins[s][voff : voff + 128 * vw].rearrange(
                                "(p f) -> p f", f=vw
                            ),
                            vst[:, 0:vw],
                        )

            def gather(s):
                nc.gpsimd.collective_compute(
                    "AllGather",
                    ALU.bypass,
                    replica_groups=REPLICA_GROUPS,
                    ins=[cc_ins[s][:].opt()],
                    outs=[cc_outs[s][:].opt()],
                )

            # sg0 and sg1 are 2-head: their single K chains interleave with
            # each other; each subgroup stages and gathers as soon as ready.
            pq01 = k_chains([0, 1])
            stage_k(0, SG[0], 0, pq01[0])
            stage_v(0, SG[0])
            gather(0)
            stage_k(1, SG[1], 0, pq01[1])
            stage_v(1, SG[1])
            gather(1)
            for s in (2, 3, 4):
                hh = SG[s]
                mb = hh[0] // 2
                pqs = k_chains([mb, mb + 1])
                for mi in range(2):
                    stage_k(s, hh, mi, pqs[mi])
                stage_v(s, hh)
                gather(s)

            # ---- Q^T projection, overlaps the AllGathers ----------------
            for mp in range(DC // 2):
                pq0 = pp.tile([128, 512], F32, tag="acc")
                pq1 = pkv.tile([128, 512], F32, tag="kvacc")
                pqs = [pq0, pq1]
                for c in range(DC):
                    for mi in range(2):
                        m = 2 * mp + mi
                        nc.tensor.matmul(
                            pqs[mi][:],
                            wq_sb[:, c * INNER + m * 128 : c * INNER + (m + 1) * 128],
                            xnt[:, c * NQ : (c + 1) * NQ],
                            start=(c == 0),
                            stop=(c == DC - 1),
                        )
                for mi in range(2):
                    m = 2 * mp + mi
                    for lh in range(2):
                        h_abs = 2 * m + lh
                        for half in range(2):
                            dst = q_t[
                                half * 64 : half * 64 + 64,
                                h_abs * NQ : (h_abs + 1) * NQ,
                            ]
                            src = pqs[mi][lh * 64 : lh * 64 + 64, :]
                            if half == 0:
                                nc.vector.tensor_copy(dst, src)
                            else:
                                nc.scalar.copy(dst, src)

        # ---- attention-phase SBUF (proj pools released) ------------------
        with ExitStack() as att_ctx:
            apool = lambda name, bufs, **kw: att_ctx.enter_context(
                tc.tile_pool(name=name, bufs=bufs, **kw)
            )
            kv = apool("kv", 1)
            wo_pool = apool("wo", 1)
            es_pool = apool("es", 18)
            rp_pool = apool("rp", 2)
            y_pool = apool("y", 2)
            ps_s = apool("ps_s", 3, space="PSUM")

            # gathered K^T duplicated per head: quarter qb, head h at cols
            # (qb*16 + h)*512, with K_h^T in both row halves (see q_t note)
            kt_sb = kv.tile([128, GROUP * HEADS * NQ], MMDT)
            # gathered V interleaved with ones blocks: chunk kc spans
            # [kc*1600, +1600): pair c = h//2 at [c*192, +192) as
            # [ones | V_{2c} | V_{2c+1}], plus a trailing ones block.
            # Head h's lhsT = cols kc*1600 + c*192 + (h%2)*128, len 128:
            # even heads [ones | V] (AV rows 0:64 = colsum, 64:128 = data),
            # odd heads [V | ones] (swapped).
            vones = kv.tile([128, KCH * VSTR], MMDT)

            for kc in range(KCH):
                ones_base = vones[:, kc * VSTR : kc * VSTR + 64]
                nc.vector.memset(
                    bass.AP(
                        tensor=ones_base.tensor,
                        offset=ones_base.offset,
                        ap=[ones_base.ap[0], [192, DC + 1], [1, 64]],
                    ),
                    1.0,
                )

            # W_out chunks are DMA'd from inside the attention loop (on the
            # Sync queue, behind the later emit_loads) so they never compete
            # with the AllGathers or the latency-critical kt/vones loads.
            wout_sb = wo_pool.tile([128, DC * DIM], MMDT)

            def emit_wout(cs):
                for c in cs:
                    nc.sync.dma_start(
                        wout_sb[:, c * DIM : (c + 1) * DIM],
                        w_out[c * 128 : (c + 1) * 128, :],
                    )

            # per subgroup loads, interleaved per quarter (K then its V) so
            # the AV pipeline isn't starved behind all the K loads;
            # subgroup 0 now, later ones interleaved with the attention loop
            def emit_loads(s):
                hh = SG[s]
                nh = len(hh)
                vw = nh * 64
                for qb in range(GROUP):
                    # K^T: heads hh merged. Half 0 comes from HBM; half 1
                    # (the K=128 duplication) is an SBUF->SBUF copy so it
                    # doesn't compete with the AllGathers for HBM.
                    ksrc = bass.AP(
                        tensor=cc_outs[s].tensor,
                        offset=cc_outs[s].offset + qb * sg_len(hh),
                        ap=[[NQ, 64], [64 * NQ, nh], [1, NQ]],
                    )
                    span = slice(
                        (qb * HEADS + hh[0]) * NQ, (qb * HEADS + hh[0] + nh) * NQ
                    )
                    nc.sync.dma_start(kt_sb[0:64, span], ksrc)
                    nc.sync.dma_start(kt_sb[64:128, span], kt_sb[0:64, span])
                    # V: this quarter's 4 key chunks, one DMA per head pair
                    for pc in range(nh // 2):
                        vdst0 = vones[
                            :, qb * 4 * VSTR + (hh[0] // 2 + pc) * 192 + 64 :
                        ]
                        nc.sync.dma_start(
                            bass.AP(
                                tensor=vdst0.tensor,
                                offset=vdst0.offset,
                                ap=[vdst0.ap[0], [VSTR, 4], [1, 128]],
                            ),
                            bass.AP(
                                tensor=cc_outs[s].tensor,
                                offset=cc_outs[s].offset
                                + qb * sg_len(hh)
                                + sg_klen(hh)
                                + pc * 128,
                                ap=[[vw, 128], [128 * vw, 4], [1, 128]],
                            ),
                        )
            emit_loads(0)

            def av_lhs(h, kc):
                base = kc * VSTR + (h // 2) * 192 + (h % 2) * 128
                return vones[:, base : base + 128]

            # ---- attention: per head, 16 key chunks in pairs -------------
            # The first two heads of each head-group run "scores-ahead":
            # all S^T/exp pairs are emitted before any AV, so the PE/ACT
            # pipeline advances while the group's V loads drain.
            def head_scores(h):
                ess = []
                for pr in range(KPAIR):
                    pss = ps_s.tile([128, 1024], F32, tag="pss", name="pss")
                    for j in range(2):
                        kc = 2 * pr + j
                        qb, t4 = kc // 4, kc % 4
                        lhs_k = kt_sb[
                            :,
                            (qb * HEADS + h) * NQ + t4 * 128 : (qb * HEADS + h) * NQ
                            + (t4 + 1) * 128,
                        ]
                        nc.tensor.matmul(
                            pss[:, j * 512 : (j + 1) * 512],
                            lhs_k,
                            q_t[:, h * NQ : (h + 1) * NQ],
                            start=True,
                            stop=True,
                        )
                    es = es_pool.tile([128, 1024], MMDT, tag="es", name="es")
                    # psum holds 2*S (duplicated operands) -> halve the scale
                    nc.scalar.activation(es[:], pss[:], AF.Exp, scale=SCALE / 2)
                    ess.append(es)
                return ess

            def head_avs(h, ess):
                po = pp.tile([128, 512], F32, tag="acc", name="po")
                for pr in range(KPAIR):
                    for j in range(2):
                        kc = 2 * pr + j
                        nc.tensor.matmul(
                            po[:],
                            av_lhs(h, kc),
                            ess[pr][:, j * 512 : (j + 1) * 512],
                            start=(pr == 0 and j == 0),
                            stop=(pr == KPAIR - 1 and j == 1),
                        )
                return po

            def head_divide(h, po):
                hp = (h % 2) * 64
                hc = h // 2
                cb, dp = hp, 64 - hp
                recip = rp_pool.tile([128, 1024], F32, tag="recip", name="recip")
                nc.vector.tensor_copy(recip[0:64, 512:1024], po[cb : cb + 64, :])
                nc.vector.reciprocal_approx_fast(
                    recip[0:64, 0:512], recip[0:64, 512:1024]
                )
                nc.vector.tensor_mul(
                    att_t[hp : hp + 64, hc * NQ : (hc + 1) * NQ],
                    po[dp : dp + 64, :],
                    recip[0:64, 0:512],
                )

            def head_full(h):
                # normal interleaved S/exp/AV pipeline for one head
                po = pp.tile([128, 512], F32, tag="acc", name="po")
                for pr in range(KPAIR):
                    pss = ps_s.tile([128, 1024], F32, tag="pss", name="pss")
                    for j in range(2):
                        kc = 2 * pr + j
                        qb, t4 = kc // 4, kc % 4
                        lhs_k = kt_sb[
                            :,
                            (qb * HEADS + h) * NQ + t4 * 128 : (qb * HEADS + h)
                            * NQ
                            + (t4 + 1) * 128,
                        ]
                        nc.tensor.matmul(
                            pss[:, j * 512 : (j + 1) * 512],
                            lhs_k,
                            q_t[:, h * NQ : (h + 1) * NQ],
                            start=True,
                            stop=True,
                        )
                    es = es_pool.tile([128, 1024], MMDT, tag="es", name="es")
                    nc.scalar.activation(es[:], pss[:], AF.Exp, scale=SCALE / 2)
                    for j in range(2):
                        kc = 2 * pr + j
                        nc.tensor.matmul(
                            po[:],
                            av_lhs(h, kc),
                            es[:, j * 512 : (j + 1) * 512],
                            start=(pr == 0 and j == 0),
                            stop=(pr == KPAIR - 1 and j == 1),
                        )
                head_divide(h, po)

            wout_done = 0
            for s, hh in enumerate(SG):
                # first two heads run "scores-ahead", covering the V loads
                h0, h1 = hh[0], hh[1]
                ess0 = head_scores(h0)
                ess1 = head_scores(h1)
                po0 = head_avs(h0, ess0)
                head_divide(h0, po0)
                po1 = head_avs(h1, ess1)
                head_divide(h1, po1)
                if s + 1 < len(SG):
                    emit_loads(s + 1)
                if s >= 1:
                    emit_wout([wout_done, wout_done + 1])
                    wout_done += 2
                for h in hh[2:]:
                    head_full(h)

            # ---- output projection y^T = W_out^T @ att^T + b_out ---------
            for m in range(DC):
                py = pp.tile([128, 512], F32, tag="acc")
                for c in range(DC):
                    nc.tensor.matmul(
                        py[:],
                        wout_sb[:, c * DIM + m * 128 : c * DIM + (m + 1) * 128],
                        att_t[:, c * NQ : (c + 1) * NQ],
                        start=(c == 0),
                        stop=(c == DC - 1),
                    )
                y_sb = y_pool.tile([128, 512], F32, tag="y")
                nc.vector.tensor_scalar(
                    out=y_sb[:],
                    in0=py[:],
                    scalar1=gbb_t[:, 16 + m : 17 + m],
                    scalar2=None,
                    op0=ALU.add,
                )
                nc.sync.dma_start(out[m * 128 : (m + 1) * 128, :], y_sb[:])

    nc.compile()
    return nc


_NC_CACHE = None


def _get_nc():
    global _NC_CACHE
    if _NC_CACHE is None:
        _NC_CACHE = build_nc()
    return _NC_CACHE


def _make_in_maps(x, ln_gamma, ln_beta, W_qk, W_v, W_out, b_out):
    mmnp = mybir.dt.np(MMDT)
    wqk = np.asarray(W_qk, dtype=np.float32)
    wqk_q = np.ascontiguousarray(wqk[:, :INNER]).astype(mmnp)
    wqk_k = np.ascontiguousarray(wqk[:, INNER:]).astype(mmnp)
    wv = np.ascontiguousarray(np.asarray(W_v, dtype=np.float32)).astype(mmnp)
    wo = np.ascontiguousarray(np.asarray(W_out, dtype=np.float32)).astype(mmnp)
    gamma = np.asarray(ln_gamma, dtype=np.float32).reshape(DC, 128).T
    beta = np.asarray(ln_beta, dtype=np.float32).reshape(DC, 128).T
    bout = np.asarray(b_out, dtype=np.float32).reshape(DC, 128).T
    gbb = np.ascontiguousarray(np.concatenate([gamma, beta, bout], axis=1))
    xf = np.asarray(x, dtype=np.float32)
    in_maps = []
    for i in range(NCORES):
        g, qq = i // GROUP, i % GROUP
        in_maps.append(
            {
                "x": np.ascontiguousarray(xf[g, qq * NQ : (qq + 1) * NQ, :]),
                "gbb": gbb,
                "wqk_k": wqk_k,
                "wqk_q": wqk_q,
                "W_v": wv,
                "W_out": wo,
            }
        )
    return in_maps


def run(inputs: dict, trace: bool = False):
    """Run the distributed kernel; returns (full_output, BassKernelResults)."""
    nc = _get_nc()
    in_maps = _make_in_maps(**inputs)
    res = run_bass_kernel_spmd(
        nc, in_maps, core_ids=list(range(NCORES)), trace=trace
    )
    out_full = np.empty((B, N, DIM), dtype=np.float32)
    for i in range(NCORES):
        g, qq = i // GROUP, i % GROUP
        out_full[g, qq * NQ : (qq + 1) * NQ, :] = res.results[i]["out"].T
    return out_full, res


def kernel(**inputs) -> np.ndarray:
    out, _ = run(inputs, trace=False)
    return out
